# revision 10
# baseline (speedup 1.0000x reference)
"""Trainium2 Bass kernel for nn_Net_vanilla_CNN (GMU local-regression + dense CNN).

Strategy (8 NeuronCores):
  Phase 1 (batch-sharded, 8 imgs/core): im2col + noise -> t = Q^T y matmul
    (fp32r) -> err -> exp activation -> PE-transpose with PSUM accumulation
    (does the 2x2 sum-pool for free) -> h1S (sum-pooled pre-BN1 features)
    + per-core partial BN1 sums.
  Host: exact global BN1 stats; folds BN1 affine into conv2 weights
    (scale into weights, bias via an appended ones-channel).
  Phase 2 (replicated, full batch on every core): conv2/3/4 + BN2-4 +
    pools + fc1 + BNfc + fc2, with exact global BN stats computed on-device
    (full batch is local, so no cross-device communication anywhere).

BN eps/affine folding identities used (all exact):
  - conv biases feeding a BN cancel (BN subtracts the mean) -> dropped.
  - a = c1*exp(-err)+c0 then avgpool then BN1  ==  BN with adjusted eps on
    sum-pooled exp(-err):  s = g1*rsqrt(var_P + 16*eps/c1^2), t = b1 - mu_P*s.
  - avgpool /4 before conv3 folded into conv3 weights.
  - maxpool pad(-inf) on relu outputs == pad(0).
"""
import math
import numpy as np

B, C_IN, H, W = 64, 1, 32, 32
K, PAD, NS, OUT_CH = 5, 2, 3, 64
P = C_IN * K * K  # 25
L = H * W  # 1024
EPS_NOISE = 1e-4
BN_EPS = 1e-5
N_CORES = 8
IMGS = B // N_CORES  # 8
C1 = 1.0 / (1.0 - math.exp(-1.0))  # exp activation scale

_cache = {}


def _host_constants(gmu_w):
    """Q matrix (25 x 257): stacked per-o orthonormal bases + ones column."""
    Xm = gmu_w.reshape(OUT_CH, P, NS).astype(np.float64)
    Xm = np.concatenate([np.ones((OUT_CH, P, 1)), Xm], axis=2)  # (64,25,4)
    cov = np.einsum("opc,opd->ocd", Xm, Xm)
    Lc = np.linalg.cholesky(cov)  # (64,4,4)
    # Q_o = Xm_o @ inv(Lc_o).T  -> orthonormal columns
    Q = np.einsum("opd,ocd->opc", Xm, np.linalg.inv(Lc))  # (64,25,4)
    qmat = np.zeros((P, 260), np.float32)
    qmat[:, :256] = Q.transpose(1, 0, 2).reshape(P, 256)
    qmat[:, 256] = 1.0  # sum column (col 257 stays 0)
    qmat[:, 259] = 1.0  # cols 258:260 = [0 | 1]: ysq-matmul rhs -> [0, sumsq]
    return qmat


def _noise_shards():
    import jax

    with jax.default_device(jax.devices("cpu")[0]):
        noise = np.asarray(
            jax.random.normal(jax.random.key(42), (B, P, L), "float32")
        ) * np.float32(EPS_NOISE)
    # per-core (25, IMGS*1024) layout [p, img*L + l]
    return [
        noise[c * IMGS:(c + 1) * IMGS].transpose(1, 0, 2).reshape(P, IMGS * L)
        for c in range(N_CORES)
    ]


def _build_phase1():
    import concourse.bacc as bacc
    import concourse.bass as bass
    import concourse.mybir as mybir
    import concourse.tile as tile

    fp32 = mybir.dt.float32
    fp32r = mybir.dt.float32r
    nc = bacc.Bacc(None, target_bir_lowering=False)
    xpad_d = nc.declare_dram_parameter("xpad", [IMGS * 1296], fp32, isOutput=False)
    noise_d = nc.declare_dram_parameter("noise", [P, IMGS * L], fp32, isOutput=False)
    qmat_d = nc.declare_dram_parameter("qmat", [P, 260], fp32, isOutput=False)
    ident_d = nc.declare_dram_parameter("ident", [128, 128], fp32, isOutput=False)
    h1S_d = nc.declare_dram_parameter("h1S", [OUT_CH, IMGS * 256], fp32, isOutput=True)
    s12_d = nc.declare_dram_parameter("s12", [OUT_CH, 2], fp32, isOutput=True)

    with tile.TileContext(nc) as tc:
        with (
            tc.tile_pool(name="const", bufs=1) as constp,
            tc.tile_pool(name="imgs", bufs=3) as imgp,
            tc.tile_pool(name="work", bufs=4) as workp,
            tc.tile_pool(name="small", bufs=8) as smallp,
            tc.tile_pool(name="out", bufs=1) as outp,
            tc.tile_pool(name="psA", bufs=4, space="PSUM") as psA,
            tc.tile_pool(name="psT", bufs=2, space="PSUM") as psT,
        ):
            qf = constp.tile([P, 260], fp32)
            nc.gpsimd.dma_start(qf[:], qmat_d[:])
            qr = constp.tile([P, 260], fp32r)
            nc.vector.tensor_copy(qr[:], qf[:])
            idt = constp.tile([128, 128], fp32)
            nc.gpsimd.dma_start(idt[:], ident_d[:])
            h1S_t = outp.tile([OUT_CH, IMGS * 256], fp32)

            for i in range(IMGS):
                y0 = imgp.tile([P, L], fp32, tag="y0")
                for ky in range(5):
                    src = bass.AP(xpad_d, i * 1296 + ky * 36,
                                  [[1, 5], [36, 32], [1, 32]])
                    nc.gpsimd.dma_start(y0[ky * 5:(ky + 1) * 5, :], src)
                nz = imgp.tile([P, L], fp32, tag="nz")
                nc.gpsimd.dma_start(nz[:], noise_d[:, i * L:(i + 1) * L])
                y2 = imgp.tile([P, L], fp32, tag="y2")
                nc.vector.tensor_add(y2[:], y0[:], nz[:])
                # permute locations to (py, px, a, b) so pooling partners sit at
                # the same position of 4 contiguous 256-blocks
                yp = imgp.tile([P, L], fp32, tag="yp")
                nc.vector.tensor_copy(
                    yp[:].rearrange("p (py px a b) -> p py px a b", py=2, px=2, a=16),
                    y2[:].rearrange("p (a py b px) -> p py px a b", py=2, b=16, px=2),
                )
                yr = imgp.tile([P, L], fp32r, tag="yr")
                nc.vector.tensor_copy(yr[:], yp[:])
                ysq = imgp.tile([P, L], fp32r, tag="ysq")
                nc.scalar.square(ysq[:], yp[:])

                for hf in range(2):
                    tp = psT.tile([OUT_CH, 128], fp32)
                    for k in range(4):
                        off = k * 256 + hf * 128
                        lhs = yr[:, off:off + 128]
                        lhs_sq = ysq[:, off:off + 128]
                        pA = psA.tile([128, 258], fp32)
                        nc.tensor.matmul(pA[:, 0:258], lhs, qr[:, 0:258],
                                         start=True, stop=False)
                        nc.tensor.matmul(pA[:, 256:258], lhs_sq, qr[:, 258:260],
                                         start=False, stop=True)
                        sc = smallp.tile([128, 2], fp32, tag="sc")
                        nc.vector.tensor_copy(sc[:], pA[:, 256:258])
                        tsq = workp.tile([128, 256], fp32, tag="tsq")
                        nc.scalar.square(tsq[:], pA[:, 0:256])
                        ssum = workp.tile([128, 64], fp32, tag="ssum")
                        nc.vector.tensor_reduce(
                            ssum[:],
                            tsq[:].rearrange("l (o c) -> l o c", c=4),
                            mybir.AxisListType.X,
                            mybir.AluOpType.add,
                        )
                        s2 = smallp.tile([128, 1], fp32, tag="s2")
                        nc.vector.tensor_mul(s2[:], sc[:, 0:1], sc[:, 0:1])
                        d = smallp.tile([128, 1], fp32, tag="d")
                        nc.vector.tensor_scalar_mul(d[:], sc[:, 1:2], 25.0)
                        nc.vector.tensor_sub(d[:], d[:], s2[:])
                        r24 = smallp.tile([128, 1], fp32, tag="r24")
                        nc.vector.reciprocal(r24[:], d[:])
                        nc.vector.tensor_scalar_mul(r24[:], r24[:], 24.0)
                        bia = smallp.tile([128, 1], fp32, tag="bia")
                        nc.vector.tensor_scalar(
                            bia[:], sc[:, 1:2], r24[:], -1.0,
                            mybir.AluOpType.mult, mybir.AluOpType.mult,
                        )
                        e = workp.tile([128, 64], fp32, tag="e")
                        nc.scalar.activation(
                            e[:], ssum[:], mybir.ActivationFunctionType.Exp,
                            bias=bia[:], scale=r24[:],
                        )
                        nc.tensor.matmul(tp[:], e[:], idt[:, 0:128],
                                         is_transpose=True,
                                         start=(k == 0), stop=(k == 3))
                    nc.vector.tensor_copy(
                        h1S_t[:, i * 256 + hf * 128:i * 256 + hf * 128 + 128], tp[:]
                    )

            s12_t = smallp.tile([OUT_CH, 2], fp32, tag="s12")
            nc.vector.tensor_reduce(
                s12_t[:, 0:1], h1S_t[:], mybir.AxisListType.X, mybir.AluOpType.add
            )
            scratch = outp.tile([OUT_CH, IMGS * 256], fp32)
            nc.scalar.activation(
                scratch[:], h1S_t[:], mybir.ActivationFunctionType.Square,
                accum_out=s12_t[:, 1:2],
            )
            nc.gpsimd.dma_start(h1S_d[:], h1S_t[:])
            nc.gpsimd.dma_start(s12_d[:], s12_t[:])
    nc.finalize()
    return nc


def _build_phase2():
    import concourse.bacc as bacc
    import concourse.bass as bass
    import concourse.mybir as mybir
    import concourse.tile as tile

    fp32 = mybir.dt.float32
    fp32r = mybir.dt.float32r
    AF = mybir.ActivationFunctionType
    AX = mybir.AxisListType
    OP = mybir.AluOpType
    nc = bacc.Bacc(None, target_bir_lowering=False)

    h1n_d = nc.declare_dram_parameter("h1n", [65, B * 324], fp32, isOutput=False)
    w2_d = nc.declare_dram_parameter("w2", [65, 9 * 128], fp32, isOutput=False)
    w3_d = nc.declare_dram_parameter("w3", [128, 9 * 128], fp32, isOutput=False)
    w4_d = nc.declare_dram_parameter("w4", [128, 9 * 128], fp32, isOutput=False)
    fc1_d = nc.declare_dram_parameter("fc1", [128, 128], fp32, isOutput=False)
    fc2_d = nc.declare_dram_parameter("fc2", [128, 10], fp32, isOutput=False)
    bn_d = nc.declare_dram_parameter("bng", [128, 8], fp32, isOutput=False)
    xm_d = nc.declare_dram_parameter("xm", [10, B], fp32, isOutput=True)
    xe_d = nc.declare_dram_parameter("xe", [128, B], fp32, isOutput=True)

    def bn_affine(tc, nc, pool, s1, s2, n, g, b, eps):
        """per-channel affine s,t from sums: s = g*rsqrt(var+eps), t = b - mu*s"""
        mu = pool.tile([128, 1], fp32, tag="bn_mu")
        nc.vector.tensor_scalar_mul(mu[:], s1, 1.0 / n)
        ve = pool.tile([128, 1], fp32, tag="bn_ve")
        nc.vector.tensor_scalar(ve[:], s2, 1.0 / n, eps, OP.mult, OP.add)
        msq = pool.tile([128, 1], fp32, tag="bn_msq")
        nc.vector.tensor_mul(msq[:], mu[:], mu[:])
        nc.vector.tensor_sub(ve[:], ve[:], msq[:])
        nc.scalar.sqrt(ve[:], ve[:])
        s = pool.tile([128, 1], fp32, tag="bn_s")
        nc.vector.reciprocal(s[:], ve[:])
        nc.vector.tensor_mul(s[:], s[:], g)
        t = pool.tile([128, 1], fp32, tag="bn_t")
        nc.vector.tensor_mul(t[:], mu[:], s[:])
        nc.vector.tensor_scalar(t[:], t[:], -1.0, None, OP.mult)
        nc.vector.tensor_add(t[:], t[:], b)
        return s, t

    with tile.TileContext(nc) as tc:
        with (
            tc.tile_pool(name="const", bufs=1) as constp,
            tc.tile_pool(name="acts", bufs=1) as actp,
            tc.tile_pool(name="stream", bufs=2) as strp,
            tc.tile_pool(name="work", bufs=1) as workp,
            tc.tile_pool(name="small", bufs=4) as smallp,
            tc.tile_pool(name="ps", bufs=4, space="PSUM") as psp,
        ):
            # ---- load + round weights
            def load_r(dram, shape):
                f = workp.tile(shape, fp32, tag="wload")
                nc.gpsimd.dma_start(f[:], dram[:])
                r = constp.tile(shape, fp32r)
                nc.vector.tensor_copy(r[:], f[:])
                return r

            w2 = load_r(w2_d, [65, 1152])
            w3 = load_r(w3_d, [128, 1152])
            w4 = load_r(w4_d, [128, 1152])
            fc1 = load_r(fc1_d, [128, 128])
            fc2 = load_r(fc2_d, [128, 10])
            bng = constp.tile([128, 8], fp32)
            nc.gpsimd.dma_start(bng[:], bn_d[:])

            # ---- conv2, streaming h1 in eighths of 8 imgs; stats fused into
            # the psum->sbuf copies via accum_out columns
            c2 = actp.tile([128, B * 256], fp32, tag="big")  # later reused
            s1c = smallp.tile([128, 32], fp32, tag="s1c")
            s2c = smallp.tile([128, 32], fp32, tag="s2c")
            sq_scr = actp.tile([128, 512], fp32, tag="sqscr")
            for e8 in range(8):
                h1f = strp.tile([65, 8 * 324], fp32, tag="h1f")
                nc.gpsimd.dma_start(h1f[:], h1n_d[:, e8 * 8 * 324:(e8 + 1) * 8 * 324])
                h1r = strp.tile([65, 8 * 324], fp32r, tag="h1r")
                nc.vector.tensor_copy(h1r[:], h1f[:])
                for gi in range(4):  # 2-img groups within the eighth
                    g = e8 * 4 + gi
                    pc = psp.tile([128, 512], fp32, tag="pc")
                    for tap in range(9):
                        ky, kx = tap // 3, tap % 3
                        rhs = bass.AP(
                            h1r.tensor, gi * 2 * 324 + ky * 18 + kx,
                            [[h1r[:].ap[0][0], 65], [324, 2], [18, 16], [1, 16]],
                        )
                        nc.tensor.matmul(
                            pc[:], w2[:, tap * 128:(tap + 1) * 128], rhs,
                            start=(tap == 0), stop=(tap == 8),
                        )
                    nc.scalar.activation(
                        c2[:, g * 512:(g + 1) * 512], pc[:], AF.Copy,
                        accum_out=s1c[:, g:g + 1],
                    )
                    nc.scalar.activation(
                        sq_scr[:], pc[:], AF.Square, accum_out=s2c[:, g:g + 1],
                    )
            c2s1 = smallp.tile([128, 1], fp32, tag="c2s1")
            c2s2 = smallp.tile([128, 1], fp32, tag="c2s2")
            nc.vector.tensor_reduce(c2s1[:], s1c[:], AX.X, OP.add)
            nc.vector.tensor_reduce(c2s2[:], s2c[:], AX.X, OP.add)
            s2a, t2a = bn_affine(tc, nc, smallp, c2s1[:], c2s2[:], B * 256,
                                 bng[:, 0:1], bng[:, 1:2], BN_EPS)
            # bn2 + relu in place
            nc.scalar.activation(c2[:], c2[:], AF.Relu, bias=t2a[:], scale=s2a[:])
            # avgpool (sum; /4 folded into w3) -> h2p padded 10x10 interior 8x8
            h2p = actp.tile([128, B * 100], fp32r, tag="h2p")
            nc.vector.memset(h2p[:].bitcast(mybir.dt.uint32), 0)
            cs = actp.tile([128, B * 128], fp32, tag="mid")  # later reused
            h2v = c2[:].rearrange("c (i y x two) -> c i y x two", i=B, y=16, two=2)
            nc.vector.tensor_add(
                cs[:].rearrange("c (i y x) -> c i y x", i=B, y=16),
                h2v[:, :, :, :, 0], h2v[:, :, :, :, 1],
            )
            cv = cs[:].rearrange("c (i y two x) -> c i y two x", i=B, y=8, two=2)
            h2pi = bass.AP(
                h2p.tensor, 11, [[h2p[:].ap[0][0], 128], [100, B], [10, 8], [1, 8]]
            )
            nc.vector.tensor_add(h2pi, cv[:, :, :, 0, :], cv[:, :, :, 1, :])

            # ---- conv3: 8x8, groups of 4 imgs (N=256)
            c3 = actp.tile([128, B * 64], fp32, tag="c3")
            for g in range(B // 4):
                pc = psp.tile([128, 256], fp32, tag="pc")
                for tap in range(9):
                    ky, kx = tap // 3, tap % 3
                    rhs = bass.AP(
                        h2p.tensor, g * 4 * 100 + ky * 10 + kx,
                        [[h2p[:].ap[0][0], 128], [100, 4], [10, 8], [1, 8]],
                    )
                    nc.tensor.matmul(
                        pc[:], w3[:, tap * 128:(tap + 1) * 128], rhs,
                        start=(tap == 0), stop=(tap == 8),
                    )
                nc.scalar.activation(
                    c3[:, g * 256:(g + 1) * 256], pc[:], AF.Copy,
                    accum_out=s1c[:, g:g + 1],
                )
                nc.scalar.activation(
                    sq_scr[:, 0:256], pc[:], AF.Square, accum_out=s2c[:, g:g + 1],
                )
            c3s1 = smallp.tile([128, 1], fp32, tag="c3s1")
            c3s2 = smallp.tile([128, 1], fp32, tag="c3s2")
            nc.vector.tensor_reduce(c3s1[:], s1c[:, 0:16], AX.X, OP.add)
            nc.vector.tensor_reduce(c3s2[:], s2c[:, 0:16], AX.X, OP.add)
            s3a, t3a = bn_affine(tc, nc, smallp, c3s1[:], c3s2[:], B * 64,
                                 bng[:, 2:3], bng[:, 3:4], BN_EPS)
            # bn3+relu into zero-padded 10x10 (relu>=0 so 0-pad == -inf pad)
            h3m = actp.tile([128, B * 100], fp32, tag="big")
            nc.vector.memset(h3m[:], 0.0)
            h3mi = bass.AP(
                h3m.tensor, 11, [[h3m[:].ap[0][0], 128], [100, B], [10, 8], [1, 8]]
            )
            nc.scalar.activation(
                h3mi, c3[:].rearrange("c (i y x) -> c i y x", i=B, y=8),
                AF.Relu, bias=t3a[:], scale=s3a[:],
            )
            # maxpool k2 s2 pad1 -> 5x5
            m1 = actp.tile([128, B * 50], fp32, tag="mid")
            h3v = h3m[:].rearrange("c (i y x two) -> c i y x two", i=B, y=10, two=2)
            nc.vector.tensor_max(
                m1[:].rearrange("c (i y x) -> c i y x", i=B, y=10),
                h3v[:, :, :, :, 0], h3v[:, :, :, :, 1],
            )
            h4p = actp.tile([128, B * 49 + 16], fp32r, tag="c3")
            nc.vector.memset(h4p[:].bitcast(mybir.dt.uint32), 0)
            m1v = m1[:].rearrange("c (i y two x) -> c i y two x", i=B, y=5, two=2)
            h4pi = bass.AP(
                h4p.tensor, 8, [[h4p[:].ap[0][0], 128], [49, B], [7, 5], [1, 5]]
            )
            nc.vector.tensor_max(h4pi, m1v[:, :, :, 0, :], m1v[:, :, :, 1, :])

            # ---- conv4: 5x5 (pad 7x7), groups of 8 imgs, 6x6 over-read (N=288)
            c4 = actp.tile([128, B * 36], fp32, tag="mid")
            for g in range(B // 8):
                pc = psp.tile([128, 288], fp32, tag="pc")
                for tap in range(9):
                    ky, kx = tap // 3, tap % 3
                    rhs = bass.AP(
                        h4p.tensor, g * 8 * 49 + ky * 7 + kx,
                        [[h4p[:].ap[0][0], 128], [49, 8], [7, 6], [1, 6]],
                    )
                    nc.tensor.matmul(
                        pc[:], w4[:, tap * 128:(tap + 1) * 128], rhs,
                        start=(tap == 0), stop=(tap == 8),
                    )
                nc.scalar.activation(c4[:, g * 288:(g + 1) * 288], pc[:], AF.Copy)
            # stats over the 5x5 valid region only
            c4v = c4[:].rearrange("c (i y x) -> c i y x", i=B, y=6)[:, :, 0:5, 0:5]
            c4s1 = smallp.tile([128, 1], fp32, tag="c4s1")
            c4s2 = smallp.tile([128, 1], fp32, tag="c4s2")
            nc.vector.tensor_reduce(c4s1[:], c4v, AX.XYZ, OP.add)
            scr4 = actp.tile([128, B * 25], fp32, tag="big")
            nc.scalar.activation(
                scr4[:].rearrange("c (i y x) -> c i y x", i=B, y=5), c4v,
                AF.Square, accum_out=c4s2[:],
            )
            s4a, t4a = bn_affine(tc, nc, smallp, c4s1[:], c4s2[:], B * 25,
                                 bng[:, 4:5], bng[:, 5:6], BN_EPS)
            h4 = actp.tile([128, B * 25], fp32, tag="h4")
            nc.scalar.activation(
                h4[:].rearrange("c (i y x) -> c i y x", i=B, y=5), c4v,
                AF.Relu, bias=t4a[:], scale=s4a[:],
            )
            # maxpool k4: max over rows 0..3 x cols 0..3 of the 5x5
            xe_t = actp.tile([128, B], fp32, tag="xe")
            h4g = h4[:].rearrange("c (i y x) -> c i y x", i=B, y=5)[:, :, 0:4, 0:4]
            nc.vector.tensor_reduce(xe_t[:], h4g, AX.XY, OP.max)
            nc.gpsimd.dma_start(xe_d[:], xe_t[:])
            xer = actp.tile([128, B], fp32r, tag="xer")
            nc.vector.tensor_copy(xer[:], xe_t[:])

            # ---- fc1 + bnfc + relu + fc2
            pf = psp.tile([128, B], fp32, tag="pc")
            nc.tensor.matmul(pf[:], fc1[:], xer[:], start=True, stop=True)
            f1 = workp.tile([128, B], fp32, tag="f1")
            fs1 = smallp.tile([128, 1], fp32, tag="fs1")
            fs2 = smallp.tile([128, 1], fp32, tag="fs2")
            nc.scalar.activation(f1[:], pf[:], AF.Copy, accum_out=fs1[:])
            scf = workp.tile([128, B], fp32, tag="scf")
            nc.scalar.activation(scf[:], f1[:], AF.Square, accum_out=fs2[:])
            sfa, tfa = bn_affine(tc, nc, smallp, fs1[:], fs2[:], B,
                                 bng[:, 6:7], bng[:, 7:8], BN_EPS)
            xfc = workp.tile([128, B], fp32r, tag="xfc")
            nc.scalar.activation(xfc[:], f1[:], AF.Relu, bias=tfa[:], scale=sfa[:])
            pm = psp.tile([10, B], fp32, tag="pc")
            nc.tensor.matmul(pm[:], fc2[:], xfc[:], start=True, stop=True)
            xm_t = workp.tile([10, B], fp32, tag="xm")
            nc.scalar.copy(xm_t[:], pm[:])
            nc.gpsimd.dma_start(xm_d[:], xm_t[:])
    nc.finalize()
    return nc


def kernel(x, gmu_w, conv2_w, conv2_b, conv3_w, conv3_b, conv4_w, conv4_b,
           fc1_w, fc1_b, fc2_w, fc2_b,
           bn1_g, bn1_b, bn2_g, bn2_b, bn3_g, bn3_b, bn4_g, bn4_b,
           bnfc_g, bnfc_b):
    from concourse.bass_utils import run_bass_kernel_spmd

    x = np.asarray(x, np.float32)
    qmat = _host_constants(np.asarray(gmu_w, np.float32))
    if "noise" not in _cache:
        _cache["noise"] = _noise_shards()
    noise_sh = _cache["noise"]

    # ---- phase 1
    if "nc1" not in _cache:
        _cache["nc1"] = _build_phase1()
    nc1 = _cache["nc1"]
    xp = np.zeros((B, 36, 36), np.float32)
    xp[:, 2:34, 2:34] = x[:, 0]
    ident = np.eye(128, dtype=np.float32)
    in_maps1 = [
        {
            "xpad": xp[c * IMGS:(c + 1) * IMGS].reshape(-1),
            "noise": noise_sh[c],
            "qmat": qmat,
            "ident": ident,
        }
        for c in range(N_CORES)
    ]
    res1 = run_bass_kernel_spmd(nc1, in_maps1, list(range(N_CORES))).results

    # ---- host: exact global BN1 stats + weight folding
    h1S = np.concatenate(
        [res1[c]["h1S"].reshape(OUT_CH, IMGS, 256) for c in range(N_CORES)], axis=1
    )  # (64, B, 256) sum-pooled exp values, spatial (y,x) 16x16
    S1 = sum(res1[c]["s12"][:, 0] for c in range(N_CORES))
    S2 = sum(res1[c]["s12"][:, 1] for c in range(N_CORES))
    nS = B * 256
    muP = S1 / nS
    varP = S2 / nS - muP * muP
    s1f = np.asarray(bn1_g) / np.sqrt(varP + 16.0 * BN_EPS / (C1 * C1))
    t1f = np.asarray(bn1_b) - muP * s1f

    # conv2 weights: fold s1 into ic columns; bias channel = conv of t1 map
    w2 = np.asarray(conv2_w, np.float32)  # (128, 64, 3, 3)
    w2f = w2 * s1f[None, :, None, None]
    tmap = np.einsum("oikl,i->okl", w2, t1f)  # (128, 3, 3)
    w2aug = np.zeros((65, 9 * 128), np.float32)
    for tap in range(9):
        ky, kx = tap // 3, tap % 3
        w2aug[:64, tap * 128:(tap + 1) * 128] = w2f[:, :, ky, kx].T
        w2aug[64, tap * 128:(tap + 1) * 128] = tmap[:, ky, kx]
    # h1n: padded 18x18 with interior h1S + ones channel (interior 1, ring 0)
    h1n = np.zeros((65, B, 18, 18), np.float32)
    h1n[:64, :, 1:17, 1:17] = h1S.reshape(OUT_CH, B, 16, 16)
    h1n[64, :, 1:17, 1:17] = 1.0

    def taps(wc):  # (oc, ic, 3, 3) -> (ic, 9*oc)
        out = np.zeros((wc.shape[1], 9 * wc.shape[0]), np.float32)
        for tap in range(9):
            ky, kx = tap // 3, tap % 3
            out[:, tap * wc.shape[0]:(tap + 1) * wc.shape[0]] = wc[:, :, ky, kx].T
        return out

    w3t = taps(np.asarray(conv3_w, np.float32) * 0.25)
    w4t = taps(np.asarray(conv4_w, np.float32))
    bng = np.stack(
        [np.asarray(v, np.float32) for v in
         (bn2_g, bn2_b, bn3_g, bn3_b, bn4_g, bn4_b, bnfc_g, bnfc_b)], axis=1
    )  # (128, 8)

    if "nc2" not in _cache:
        _cache["nc2"] = _build_phase2()
    nc2 = _cache["nc2"]
    im2 = {
        "h1n": h1n.reshape(65, B * 324),
        "w2": w2aug,
        "w3": w3t,
        "w4": w4t,
        "fc1": np.asarray(fc1_w, np.float32).reshape(128, 128).T.copy(),
        "fc2": np.asarray(fc2_w, np.float32).reshape(10, 128).T.copy(),
        "bng": bng,
    }
    res2 = run_bass_kernel_spmd(nc2, [im2] * N_CORES, list(range(N_CORES))).results

    xm = res2[0]["xm"].T.copy() + np.asarray(fc2_b, np.float32)[None, :]  # (B, 10)
    xe = res2[0]["xe"].T.reshape(B, 128, 1, 1).copy()
    return xm, xe


# revision 13
# speedup vs baseline: 1.0289x; 1.0289x over previous
"""Trainium2 Bass kernel for nn_Net_vanilla_CNN (GMU local-regression + dense CNN).

Strategy (8 NeuronCores):
  Phase 1 (batch-sharded, 8 imgs/core): im2col + noise -> t = Q^T y matmul
    (fp32r) -> err -> exp activation -> PE-transpose with PSUM accumulation
    (does the 2x2 sum-pool for free) -> h1S (sum-pooled pre-BN1 features)
    + per-core partial BN1 sums.
  Host: exact global BN1 stats; folds BN1 affine into conv2 weights
    (scale into weights, bias via an appended ones-channel).
  Phase 2 (replicated, full batch on every core): conv2/3/4 + BN2-4 +
    pools + fc1 + BNfc + fc2, with exact global BN stats computed on-device
    (full batch is local, so no cross-device communication anywhere).

BN eps/affine folding identities used (all exact):
  - conv biases feeding a BN cancel (BN subtracts the mean) -> dropped.
  - a = c1*exp(-err)+c0 then avgpool then BN1  ==  BN with adjusted eps on
    sum-pooled exp(-err):  s = g1*rsqrt(var_P + 16*eps/c1^2), t = b1 - mu_P*s.
  - avgpool /4 before conv3 folded into conv3 weights.
  - maxpool pad(-inf) on relu outputs == pad(0).
"""
import math
import numpy as np

B, C_IN, H, W = 64, 1, 32, 32
K, PAD, NS, OUT_CH = 5, 2, 3, 64
P = C_IN * K * K  # 25
L = H * W  # 1024
EPS_NOISE = 1e-4
BN_EPS = 1e-5
N_CORES = 8
IMGS = B // N_CORES  # 8
C1 = 1.0 / (1.0 - math.exp(-1.0))  # exp activation scale

_cache = {}


def _host_constants(gmu_w):
    """Q matrix (25 x 257): stacked per-o orthonormal bases + ones column."""
    Xm = gmu_w.reshape(OUT_CH, P, NS).astype(np.float64)
    Xm = np.concatenate([np.ones((OUT_CH, P, 1)), Xm], axis=2)  # (64,25,4)
    cov = np.einsum("opc,opd->ocd", Xm, Xm)
    Lc = np.linalg.cholesky(cov)  # (64,4,4)
    # Q_o = Xm_o @ inv(Lc_o).T  -> orthonormal columns
    Q = np.einsum("opd,ocd->opc", Xm, np.linalg.inv(Lc))  # (64,25,4)
    qmat = np.zeros((P, 260), np.float32)
    qmat[:, :256] = Q.transpose(1, 0, 2).reshape(P, 256)
    qmat[:, 256] = 1.0  # sum column (col 257 stays 0)
    qmat[:, 259] = 1.0  # cols 258:260 = [0 | 1]: ysq-matmul rhs -> [0, sumsq]
    return qmat


def _noise_shards():
    import jax

    with jax.default_device(jax.devices("cpu")[0]):
        noise = np.asarray(
            jax.random.normal(jax.random.key(42), (B, P, L), "float32")
        ) * np.float32(EPS_NOISE)
    # per-core (25, IMGS*1024) layout [p, img*L + l]
    return [
        noise[c * IMGS:(c + 1) * IMGS].transpose(1, 0, 2).reshape(P, IMGS * L)
        for c in range(N_CORES)
    ]


def _build_phase1():
    import concourse.bacc as bacc
    import concourse.bass as bass
    import concourse.mybir as mybir
    import concourse.tile as tile

    fp32 = mybir.dt.float32
    fp32r = mybir.dt.float32r
    nc = bacc.Bacc(None, target_bir_lowering=False)
    xpad_d = nc.declare_dram_parameter("xpad", [IMGS * 1296], fp32, isOutput=False)
    noise_d = nc.declare_dram_parameter("noise", [P, IMGS * L], fp32, isOutput=False)
    qmat_d = nc.declare_dram_parameter("qmat", [P, 260], fp32, isOutput=False)
    ident_d = nc.declare_dram_parameter("ident", [128, 128], fp32, isOutput=False)
    h1S_d = nc.declare_dram_parameter("h1S", [OUT_CH, IMGS * 256], fp32, isOutput=True)
    s12_d = nc.declare_dram_parameter("s12", [OUT_CH, 2], fp32, isOutput=True)

    with tile.TileContext(nc) as tc:
        with (
            tc.tile_pool(name="const", bufs=1) as constp,
            tc.tile_pool(name="imgs", bufs=3) as imgp,
            tc.tile_pool(name="work", bufs=4) as workp,
            tc.tile_pool(name="small", bufs=8) as smallp,
            tc.tile_pool(name="out", bufs=1) as outp,
            tc.tile_pool(name="psA", bufs=4, space="PSUM") as psA,
            tc.tile_pool(name="psT", bufs=2, space="PSUM") as psT,
        ):
            qf = constp.tile([P, 260], fp32)
            nc.gpsimd.dma_start(qf[:], qmat_d[:])
            qr = constp.tile([P, 260], fp32r)
            nc.vector.tensor_copy(qr[:], qf[:])
            idt = constp.tile([128, 128], fp32)
            nc.gpsimd.dma_start(idt[:], ident_d[:])
            h1S_t = outp.tile([OUT_CH, IMGS * 256], fp32)

            for i in range(IMGS):
                y0 = imgp.tile([P, L], fp32, tag="y0")
                for ky in range(5):
                    src = bass.AP(xpad_d, i * 1296 + ky * 36,
                                  [[1, 5], [36, 32], [1, 32]])
                    nc.gpsimd.dma_start(y0[ky * 5:(ky + 1) * 5, :], src)
                nz = imgp.tile([P, L], fp32, tag="nz")
                nc.gpsimd.dma_start(nz[:], noise_d[:, i * L:(i + 1) * L])
                y2 = imgp.tile([P, L], fp32, tag="y2")
                nc.vector.tensor_add(y2[:], y0[:], nz[:])
                # permute locations to (py, px, a, b) so pooling partners sit at
                # the same position of 4 contiguous 256-blocks
                yp = imgp.tile([P, L], fp32, tag="yp")
                nc.vector.tensor_copy(
                    yp[:].rearrange("p (py px a b) -> p py px a b", py=2, px=2, a=16),
                    y2[:].rearrange("p (a py b px) -> p py px a b", py=2, b=16, px=2),
                )
                yr = imgp.tile([P, L], fp32r, tag="yr")
                nc.vector.tensor_copy(yr[:], yp[:])
                ysq = imgp.tile([P, L], fp32r, tag="ysq")
                nc.scalar.square(ysq[:], yp[:])

                for hf in range(2):
                    tp = psT.tile([OUT_CH, 128], fp32)
                    for k in range(4):
                        off = k * 256 + hf * 128
                        lhs = yr[:, off:off + 128]
                        lhs_sq = ysq[:, off:off + 128]
                        pA = psA.tile([128, 258], fp32)
                        nc.tensor.matmul(pA[:, 0:258], lhs, qr[:, 0:258],
                                         start=True, stop=False)
                        nc.tensor.matmul(pA[:, 256:258], lhs_sq, qr[:, 258:260],
                                         start=False, stop=True)
                        sc = smallp.tile([128, 2], fp32, tag="sc")
                        nc.vector.tensor_copy(sc[:], pA[:, 256:258])
                        tsq = workp.tile([128, 256], fp32, tag="tsq")
                        nc.scalar.square(tsq[:], pA[:, 0:256])
                        ssum = workp.tile([128, 64], fp32, tag="ssum")
                        nc.vector.tensor_reduce(
                            ssum[:],
                            tsq[:].rearrange("l (o c) -> l o c", c=4),
                            mybir.AxisListType.X,
                            mybir.AluOpType.add,
                        )
                        s2 = smallp.tile([128, 1], fp32, tag="s2")
                        nc.vector.tensor_mul(s2[:], sc[:, 0:1], sc[:, 0:1])
                        d = smallp.tile([128, 1], fp32, tag="d")
                        nc.vector.tensor_scalar_mul(d[:], sc[:, 1:2], 25.0)
                        nc.vector.tensor_sub(d[:], d[:], s2[:])
                        r24 = smallp.tile([128, 1], fp32, tag="r24")
                        nc.vector.reciprocal(r24[:], d[:])
                        nc.vector.tensor_scalar_mul(r24[:], r24[:], 24.0)
                        bia = smallp.tile([128, 1], fp32, tag="bia")
                        nc.vector.tensor_scalar(
                            bia[:], sc[:, 1:2], r24[:], -1.0,
                            mybir.AluOpType.mult, mybir.AluOpType.mult,
                        )
                        e = workp.tile([128, 64], fp32, tag="e")
                        nc.scalar.activation(
                            e[:], ssum[:], mybir.ActivationFunctionType.Exp,
                            bias=bia[:], scale=r24[:],
                        )
                        nc.tensor.matmul(tp[:], e[:], idt[:, 0:128],
                                         is_transpose=True,
                                         start=(k == 0), stop=(k == 3))
                    nc.vector.tensor_copy(
                        h1S_t[:, i * 256 + hf * 128:i * 256 + hf * 128 + 128], tp[:]
                    )

            s12_t = smallp.tile([OUT_CH, 2], fp32, tag="s12")
            nc.vector.tensor_reduce(
                s12_t[:, 0:1], h1S_t[:], mybir.AxisListType.X, mybir.AluOpType.add
            )
            scratch = outp.tile([OUT_CH, IMGS * 256], fp32)
            nc.scalar.activation(
                scratch[:], h1S_t[:], mybir.ActivationFunctionType.Square,
                accum_out=s12_t[:, 1:2],
            )
            nc.gpsimd.dma_start(h1S_d[:], h1S_t[:])
            nc.gpsimd.dma_start(s12_d[:], s12_t[:])
    nc.finalize()
    return nc


def _build_phase2():
    import concourse.bacc as bacc
    import concourse.bass as bass
    import concourse.mybir as mybir
    import concourse.tile as tile

    fp32 = mybir.dt.float32
    fp32r = mybir.dt.float32r
    AF = mybir.ActivationFunctionType
    AX = mybir.AxisListType
    OP = mybir.AluOpType
    nc = bacc.Bacc(None, target_bir_lowering=False)

    h1n_d = nc.declare_dram_parameter("h1n", [65, B * 324], fp32, isOutput=False)
    w2_d = nc.declare_dram_parameter("w2", [65, 9 * 128], fp32, isOutput=False)
    w3_d = nc.declare_dram_parameter("w3", [128, 9 * 128], fp32, isOutput=False)
    w4_d = nc.declare_dram_parameter("w4", [128, 9 * 128], fp32, isOutput=False)
    fc1_d = nc.declare_dram_parameter("fc1", [128, 128], fp32, isOutput=False)
    fc2_d = nc.declare_dram_parameter("fc2", [128, 10], fp32, isOutput=False)
    bn_d = nc.declare_dram_parameter("bng", [128, 8], fp32, isOutput=False)
    xm_d = nc.declare_dram_parameter("xm", [10, B], fp32, isOutput=True)
    xe_d = nc.declare_dram_parameter("xe", [128, B], fp32, isOutput=True)

    def bn_affine(tc, nc, pool, s1, s2, n, g, b, eps):
        """per-channel affine s,t from sums: s = g*rsqrt(var+eps), t = b - mu*s"""
        mu = pool.tile([128, 1], fp32, tag="bn_mu")
        nc.vector.tensor_scalar_mul(mu[:], s1, 1.0 / n)
        ve = pool.tile([128, 1], fp32, tag="bn_ve")
        nc.vector.tensor_scalar(ve[:], s2, 1.0 / n, eps, OP.mult, OP.add)
        msq = pool.tile([128, 1], fp32, tag="bn_msq")
        nc.vector.tensor_mul(msq[:], mu[:], mu[:])
        nc.vector.tensor_sub(ve[:], ve[:], msq[:])
        nc.scalar.sqrt(ve[:], ve[:])
        s = pool.tile([128, 1], fp32, tag="bn_s")
        nc.vector.reciprocal(s[:], ve[:])
        nc.vector.tensor_mul(s[:], s[:], g)
        t = pool.tile([128, 1], fp32, tag="bn_t")
        nc.vector.tensor_mul(t[:], mu[:], s[:])
        nc.vector.tensor_scalar(t[:], t[:], -1.0, None, OP.mult)
        nc.vector.tensor_add(t[:], t[:], b)
        return s, t

    with tile.TileContext(nc) as tc:
        with (
            tc.tile_pool(name="const", bufs=1) as constp,
            tc.tile_pool(name="acts", bufs=1) as actp,
            tc.tile_pool(name="stream", bufs=2) as strp,
            tc.tile_pool(name="work", bufs=1) as workp,
            tc.tile_pool(name="small", bufs=4) as smallp,
            tc.tile_pool(name="ps", bufs=4, space="PSUM") as psp,
        ):
            # ---- load + round weights
            def load_r(dram, shape):
                f = workp.tile(shape, fp32, tag="wload")
                nc.gpsimd.dma_start(f[:], dram[:])
                r = constp.tile(shape, fp32r)
                nc.vector.tensor_copy(r[:], f[:])
                return r

            w2 = load_r(w2_d, [65, 1152])
            w3 = load_r(w3_d, [128, 1152])
            w4 = load_r(w4_d, [128, 1152])
            fc1 = load_r(fc1_d, [128, 128])
            fc2 = load_r(fc2_d, [128, 10])
            bng = constp.tile([128, 8], fp32)
            nc.gpsimd.dma_start(bng[:], bn_d[:])

            # ---- conv2, streaming h1 in eighths of 8 imgs; stats fused into
            # the psum->sbuf copies via accum_out columns
            c2 = actp.tile([128, B * 256], fp32, tag="big")  # later reused
            s1c = smallp.tile([128, 32], fp32, tag="s1c")
            s2c = smallp.tile([128, 32], fp32, tag="s2c")
            sq_scr = actp.tile([128, 512], fp32, tag="sqscr")
            for e8 in range(8):
                h1f = strp.tile([65, 8 * 324], fp32, tag="h1f")
                nc.gpsimd.dma_start(h1f[:], h1n_d[:, e8 * 8 * 324:(e8 + 1) * 8 * 324])
                h1r = strp.tile([65, 8 * 324], fp32r, tag="h1r")
                nc.vector.tensor_copy(h1r[:], h1f[:])
                for gi in range(4):  # 2-img groups within the eighth
                    g = e8 * 4 + gi
                    pc = psp.tile([128, 512], fp32, tag="pc")
                    for tap in range(9):
                        ky, kx = tap // 3, tap % 3
                        rhs = bass.AP(
                            h1r.tensor, gi * 2 * 324 + ky * 18 + kx,
                            [[h1r[:].ap[0][0], 65], [324, 2], [18, 16], [1, 16]],
                        )
                        nc.tensor.matmul(
                            pc[:], w2[:, tap * 128:(tap + 1) * 128], rhs,
                            start=(tap == 0), stop=(tap == 8),
                        )
                    nc.scalar.activation(
                        c2[:, g * 512:(g + 1) * 512], pc[:], AF.Copy,
                        accum_out=s1c[:, g:g + 1],
                    )
                    nc.scalar.activation(
                        sq_scr[:], pc[:], AF.Square, accum_out=s2c[:, g:g + 1],
                    )
            c2s1 = smallp.tile([128, 1], fp32, tag="c2s1")
            c2s2 = smallp.tile([128, 1], fp32, tag="c2s2")
            nc.vector.tensor_reduce(c2s1[:], s1c[:], AX.X, OP.add)
            nc.vector.tensor_reduce(c2s2[:], s2c[:], AX.X, OP.add)
            s2a, t2a = bn_affine(tc, nc, smallp, c2s1[:], c2s2[:], B * 256,
                                 bng[:, 0:1], bng[:, 1:2], BN_EPS)
            # bn2 + relu in place
            nc.scalar.activation(c2[:], c2[:], AF.Relu, bias=t2a[:], scale=s2a[:])
            # avgpool (sum; /4 folded into w3) -> h2p padded 10x10 interior 8x8
            h2p = actp.tile([128, B * 100], fp32r, tag="h2p")
            nc.vector.memset(h2p[:].bitcast(mybir.dt.uint32), 0)
            cs = actp.tile([128, B * 128], fp32, tag="mid")  # later reused
            h2v = c2[:].rearrange("c (i y x two) -> c i y x two", i=B, y=16, two=2)
            nc.vector.tensor_add(
                cs[:].rearrange("c (i y x) -> c i y x", i=B, y=16),
                h2v[:, :, :, :, 0], h2v[:, :, :, :, 1],
            )
            cv = cs[:].rearrange("c (i y two x) -> c i y two x", i=B, y=8, two=2)
            h2pi = bass.AP(
                h2p.tensor, 11, [[h2p[:].ap[0][0], 128], [100, B], [10, 8], [1, 8]]
            )
            nc.vector.tensor_add(h2pi, cv[:, :, :, 0, :], cv[:, :, :, 1, :])

            # ---- conv3: 8x8, groups of 4 imgs (N=256)
            c3 = actp.tile([128, B * 64], fp32, tag="c3")
            for g in range(B // 4):
                pc = psp.tile([128, 256], fp32, tag="pc")
                for tap in range(9):
                    ky, kx = tap // 3, tap % 3
                    rhs = bass.AP(
                        h2p.tensor, g * 4 * 100 + ky * 10 + kx,
                        [[h2p[:].ap[0][0], 128], [100, 4], [10, 8], [1, 8]],
                    )
                    nc.tensor.matmul(
                        pc[:], w3[:, tap * 128:(tap + 1) * 128], rhs,
                        start=(tap == 0), stop=(tap == 8),
                    )
                nc.scalar.activation(
                    c3[:, g * 256:(g + 1) * 256], pc[:], AF.Copy,
                    accum_out=s1c[:, g:g + 1],
                )
                nc.scalar.activation(
                    sq_scr[:, 0:256], pc[:], AF.Square, accum_out=s2c[:, g:g + 1],
                )
            c3s1 = smallp.tile([128, 1], fp32, tag="c3s1")
            c3s2 = smallp.tile([128, 1], fp32, tag="c3s2")
            nc.vector.tensor_reduce(c3s1[:], s1c[:, 0:16], AX.X, OP.add)
            nc.vector.tensor_reduce(c3s2[:], s2c[:, 0:16], AX.X, OP.add)
            s3a, t3a = bn_affine(tc, nc, smallp, c3s1[:], c3s2[:], B * 64,
                                 bng[:, 2:3], bng[:, 3:4], BN_EPS)
            # bn3+relu into zero-padded 10x10 (relu>=0 so 0-pad == -inf pad)
            h3m = actp.tile([128, B * 100], fp32, tag="big")
            nc.vector.memset(h3m[:], 0.0)
            h3mi = bass.AP(
                h3m.tensor, 11, [[h3m[:].ap[0][0], 128], [100, B], [10, 8], [1, 8]]
            )
            nc.scalar.activation(
                h3mi, c3[:].rearrange("c (i y x) -> c i y x", i=B, y=8),
                AF.Relu, bias=t3a[:], scale=s3a[:],
            )
            # maxpool k2 s2 pad1 -> 5x5
            m1 = actp.tile([128, B * 50], fp32, tag="mid")
            h3v = h3m[:].rearrange("c (i y x two) -> c i y x two", i=B, y=10, two=2)
            nc.vector.tensor_max(
                m1[:].rearrange("c (i y x) -> c i y x", i=B, y=10),
                h3v[:, :, :, :, 0], h3v[:, :, :, :, 1],
            )
            h4p = actp.tile([128, B * 49 + 16], fp32r, tag="c3")
            nc.vector.memset(h4p[:].bitcast(mybir.dt.uint32), 0)
            m1v = m1[:].rearrange("c (i y two x) -> c i y two x", i=B, y=5, two=2)
            h4pi = bass.AP(
                h4p.tensor, 8, [[h4p[:].ap[0][0], 128], [49, B], [7, 5], [1, 5]]
            )
            nc.vector.tensor_max(h4pi, m1v[:, :, :, 0, :], m1v[:, :, :, 1, :])

            # ---- conv4: 5x5 (pad 7x7), groups of 8 imgs, 6x6 over-read (N=288)
            c4 = actp.tile([128, B * 36], fp32, tag="mid")
            for g in range(B // 8):
                pc = psp.tile([128, 288], fp32, tag="pc")
                for tap in range(9):
                    ky, kx = tap // 3, tap % 3
                    rhs = bass.AP(
                        h4p.tensor, g * 8 * 49 + ky * 7 + kx,
                        [[h4p[:].ap[0][0], 128], [49, 8], [7, 6], [1, 6]],
                    )
                    nc.tensor.matmul(
                        pc[:], w4[:, tap * 128:(tap + 1) * 128], rhs,
                        start=(tap == 0), stop=(tap == 8),
                    )
                nc.scalar.activation(c4[:, g * 288:(g + 1) * 288], pc[:], AF.Copy)
            # stats over the 5x5 valid region only
            c4v = c4[:].rearrange("c (i y x) -> c i y x", i=B, y=6)[:, :, 0:5, 0:5]
            c4s1 = smallp.tile([128, 1], fp32, tag="c4s1")
            c4s2 = smallp.tile([128, 1], fp32, tag="c4s2")
            nc.vector.tensor_reduce(c4s1[:], c4v, AX.XYZ, OP.add)
            scr4 = actp.tile([128, B * 25], fp32, tag="big")
            nc.scalar.activation(
                scr4[:].rearrange("c (i y x) -> c i y x", i=B, y=5), c4v,
                AF.Square, accum_out=c4s2[:],
            )
            s4a, t4a = bn_affine(tc, nc, smallp, c4s1[:], c4s2[:], B * 25,
                                 bng[:, 4:5], bng[:, 5:6], BN_EPS)
            h4 = actp.tile([128, B * 25], fp32, tag="h4")
            nc.scalar.activation(
                h4[:].rearrange("c (i y x) -> c i y x", i=B, y=5), c4v,
                AF.Relu, bias=t4a[:], scale=s4a[:],
            )
            # maxpool k4: max over rows 0..3 x cols 0..3 of the 5x5
            xe_t = actp.tile([128, B], fp32, tag="xe")
            h4g = h4[:].rearrange("c (i y x) -> c i y x", i=B, y=5)[:, :, 0:4, 0:4]
            nc.vector.tensor_reduce(xe_t[:], h4g, AX.XY, OP.max)
            nc.gpsimd.dma_start(xe_d[:], xe_t[:])
            xer = actp.tile([128, B], fp32r, tag="xer")
            nc.vector.tensor_copy(xer[:], xe_t[:])

            # ---- fc1 + bnfc + relu + fc2
            pf = psp.tile([128, B], fp32, tag="pc")
            nc.tensor.matmul(pf[:], fc1[:], xer[:], start=True, stop=True)
            f1 = workp.tile([128, B], fp32, tag="f1")
            fs1 = smallp.tile([128, 1], fp32, tag="fs1")
            fs2 = smallp.tile([128, 1], fp32, tag="fs2")
            nc.scalar.activation(f1[:], pf[:], AF.Copy, accum_out=fs1[:])
            scf = workp.tile([128, B], fp32, tag="scf")
            nc.scalar.activation(scf[:], f1[:], AF.Square, accum_out=fs2[:])
            sfa, tfa = bn_affine(tc, nc, smallp, fs1[:], fs2[:], B,
                                 bng[:, 6:7], bng[:, 7:8], BN_EPS)
            xfc = workp.tile([128, B], fp32r, tag="xfc")
            nc.scalar.activation(xfc[:], f1[:], AF.Relu, bias=tfa[:], scale=sfa[:])
            pm = psp.tile([10, B], fp32, tag="pc")
            nc.tensor.matmul(pm[:], fc2[:], xfc[:], start=True, stop=True)
            xm_t = workp.tile([10, B], fp32, tag="xm")
            nc.scalar.copy(xm_t[:], pm[:])
            nc.gpsimd.dma_start(xm_d[:], xm_t[:])
    nc.finalize()
    return nc


def kernel(x, gmu_w, conv2_w, conv2_b, conv3_w, conv3_b, conv4_w, conv4_b,
           fc1_w, fc1_b, fc2_w, fc2_b,
           bn1_g, bn1_b, bn2_g, bn2_b, bn3_g, bn3_b, bn4_g, bn4_b,
           bnfc_g, bnfc_b):
    from concourse.bass_utils import run_bass_kernel_spmd

    x = np.asarray(x, np.float32)
    qmat = _host_constants(np.asarray(gmu_w, np.float32))
    if "noise" not in _cache:
        _cache["noise"] = _noise_shards()
    noise_sh = _cache["noise"]

    # ---- phase 1
    if "nc1" not in _cache:
        _cache["nc1"] = _build_phase1()
    nc1 = _cache["nc1"]
    xp = np.zeros((B, 36, 36), np.float32)
    xp[:, 2:34, 2:34] = x[:, 0]
    ident = np.eye(128, dtype=np.float32)
    in_maps1 = [
        {
            "xpad": xp[c * IMGS:(c + 1) * IMGS].reshape(-1),
            "noise": noise_sh[c],
            "qmat": qmat,
            "ident": ident,
        }
        for c in range(N_CORES)
    ]
    res1 = run_bass_kernel_spmd(nc1, in_maps1, list(range(N_CORES))).results

    # ---- host: exact global BN1 stats + weight folding
    h1S = np.concatenate(
        [res1[c]["h1S"].reshape(OUT_CH, IMGS, 256) for c in range(N_CORES)], axis=1
    )  # (64, B, 256) sum-pooled exp values, spatial (y,x) 16x16
    S1 = sum(res1[c]["s12"][:, 0] for c in range(N_CORES))
    S2 = sum(res1[c]["s12"][:, 1] for c in range(N_CORES))
    nS = B * 256
    muP = S1 / nS
    varP = S2 / nS - muP * muP
    s1f = np.asarray(bn1_g) / np.sqrt(varP + 16.0 * BN_EPS / (C1 * C1))
    t1f = np.asarray(bn1_b) - muP * s1f

    # conv2 weights: fold s1 into ic columns; bias channel = conv of t1 map
    w2 = np.asarray(conv2_w, np.float32)  # (128, 64, 3, 3)
    w2f = w2 * s1f[None, :, None, None]
    tmap = np.einsum("oikl,i->okl", w2, t1f)  # (128, 3, 3)
    w2aug = np.zeros((65, 9 * 128), np.float32)
    for tap in range(9):
        ky, kx = tap // 3, tap % 3
        w2aug[:64, tap * 128:(tap + 1) * 128] = w2f[:, :, ky, kx].T
        w2aug[64, tap * 128:(tap + 1) * 128] = tmap[:, ky, kx]
    # h1n: padded 18x18 with interior h1S + ones channel (interior 1, ring 0)
    h1n = np.zeros((65, B, 18, 18), np.float32)
    h1n[:64, :, 1:17, 1:17] = h1S.reshape(OUT_CH, B, 16, 16)
    h1n[64, :, 1:17, 1:17] = 1.0

    def taps(wc):  # (oc, ic, 3, 3) -> (ic, 9*oc)
        out = np.zeros((wc.shape[1], 9 * wc.shape[0]), np.float32)
        for tap in range(9):
            ky, kx = tap // 3, tap % 3
            out[:, tap * wc.shape[0]:(tap + 1) * wc.shape[0]] = wc[:, :, ky, kx].T
        return out

    w3t = taps(np.asarray(conv3_w, np.float32) * 0.25)
    w4t = taps(np.asarray(conv4_w, np.float32))
    bng = np.stack(
        [np.asarray(v, np.float32) for v in
         (bn2_g, bn2_b, bn3_g, bn3_b, bn4_g, bn4_b, bnfc_g, bnfc_b)], axis=1
    )  # (128, 8)

    if "nc2" not in _cache:
        _cache["nc2"] = _build_phase2()
    nc2 = _cache["nc2"]
    im2 = {
        "h1n": h1n.reshape(65, B * 324),
        "w2": w2aug,
        "w3": w3t,
        "w4": w4t,
        "fc1": np.asarray(fc1_w, np.float32).reshape(128, 128).T.copy(),
        "fc2": np.asarray(fc2_w, np.float32).reshape(10, 128).T.copy(),
        "bng": bng,
    }
    res2 = run_bass_kernel_spmd(nc2, [im2] * N_CORES, list(range(N_CORES))).results

    xm = res2[0]["xm"].T.copy() + np.asarray(fc2_b, np.float32)[None, :]  # (B, 10)
    xe = res2[0]["xe"].T.reshape(B, 128, 1, 1).copy()
    return xm, xe


# revision 15
# speedup vs baseline: 1.0663x; 1.0364x over previous
"""Trainium2 Bass kernel for nn_Net_vanilla_CNN (GMU local-regression + dense CNN).

Strategy (8 NeuronCores):
  Phase 1 (batch-sharded, 8 imgs/core): im2col + noise -> t = Q^T y matmul
    (fp32r) -> err -> exp activation -> PE-transpose with PSUM accumulation
    (does the 2x2 sum-pool for free) -> h1S (sum-pooled pre-BN1 features)
    + per-core partial BN1 sums.
  Host: exact global BN1 stats; folds BN1 affine into conv2 weights
    (scale into weights, bias via an appended ones-channel).
  Phase 2 (replicated, full batch on every core): conv2/3/4 + BN2-4 +
    pools + fc1 + BNfc + fc2, with exact global BN stats computed on-device
    (full batch is local, so no cross-device communication anywhere).

BN eps/affine folding identities used (all exact):
  - conv biases feeding a BN cancel (BN subtracts the mean) -> dropped.
  - a = c1*exp(-err)+c0 then avgpool then BN1  ==  BN with adjusted eps on
    sum-pooled exp(-err):  s = g1*rsqrt(var_P + 16*eps/c1^2), t = b1 - mu_P*s.
  - avgpool /4 before conv3 folded into conv3 weights.
  - maxpool pad(-inf) on relu outputs == pad(0).
"""
import math
import numpy as np

B, C_IN, H, W = 64, 1, 32, 32
K, PAD, NS, OUT_CH = 5, 2, 3, 64
P = C_IN * K * K  # 25
L = H * W  # 1024
EPS_NOISE = 1e-4
BN_EPS = 1e-5
N_CORES = 8
IMGS = B // N_CORES  # 8
C1 = 1.0 / (1.0 - math.exp(-1.0))  # exp activation scale

_cache = {}


def _host_constants(gmu_w):
    """Q matrix (25 x 257): stacked per-o orthonormal bases + ones column."""
    Xm = gmu_w.reshape(OUT_CH, P, NS).astype(np.float64)
    Xm = np.concatenate([np.ones((OUT_CH, P, 1)), Xm], axis=2)  # (64,25,4)
    cov = np.einsum("opc,opd->ocd", Xm, Xm)
    Lc = np.linalg.cholesky(cov)  # (64,4,4)
    # Q_o = Xm_o @ inv(Lc_o).T  -> orthonormal columns
    Q = np.einsum("opd,ocd->opc", Xm, np.linalg.inv(Lc))  # (64,25,4)
    qmat = np.zeros((P, 260), np.float32)
    qmat[:, :256] = Q.transpose(1, 0, 2).reshape(P, 256)
    qmat[:, 256] = 1.0  # sum column (col 257 stays 0)
    qmat[:, 259] = 1.0  # cols 258:260 = [0 | 1]: ysq-matmul rhs -> [0, sumsq]
    return qmat


def _noise_shards():
    import jax

    with jax.default_device(jax.devices("cpu")[0]):
        noise = np.asarray(
            jax.random.normal(jax.random.key(42), (B, P, L), "float32")
        ) * np.float32(EPS_NOISE)
    # per-core (25, IMGS*1024) layout [p, img*L + l]
    return [
        noise[c * IMGS:(c + 1) * IMGS].transpose(1, 0, 2).reshape(P, IMGS * L)
        for c in range(N_CORES)
    ]


def _build_phase1():
    import concourse.bacc as bacc
    import concourse.bass as bass
    import concourse.mybir as mybir
    import concourse.tile as tile

    fp32 = mybir.dt.float32
    fp32r = mybir.dt.float32r
    nc = bacc.Bacc(None, target_bir_lowering=False)
    xpad_d = nc.declare_dram_parameter("xpad", [IMGS * 1296], fp32, isOutput=False)
    noise_d = nc.declare_dram_parameter("noise", [P, IMGS * L], fp32, isOutput=False)
    qmat_d = nc.declare_dram_parameter("qmat", [P, 260], fp32, isOutput=False)
    ident_d = nc.declare_dram_parameter("ident", [128, 128], fp32, isOutput=False)
    h1S_d = nc.declare_dram_parameter("h1S", [OUT_CH, IMGS * 256], fp32, isOutput=True)
    s12_d = nc.declare_dram_parameter("s12", [OUT_CH, 2], fp32, isOutput=True)

    with tile.TileContext(nc) as tc:
        with (
            tc.tile_pool(name="const", bufs=1) as constp,
            tc.tile_pool(name="imgs", bufs=3) as imgp,
            tc.tile_pool(name="work", bufs=4) as workp,
            tc.tile_pool(name="small", bufs=8) as smallp,
            tc.tile_pool(name="out", bufs=1) as outp,
            tc.tile_pool(name="psA", bufs=4, space="PSUM") as psA,
            tc.tile_pool(name="psT", bufs=2, space="PSUM") as psT,
        ):
            qf = constp.tile([P, 260], fp32)
            nc.gpsimd.dma_start(qf[:], qmat_d[:])
            qr = constp.tile([P, 260], fp32r)
            nc.vector.tensor_copy(qr[:], qf[:])
            idt = constp.tile([128, 128], fp32)
            nc.gpsimd.dma_start(idt[:], ident_d[:])
            h1S_t = outp.tile([OUT_CH, IMGS * 256], fp32)

            for i in range(IMGS):
                y0 = imgp.tile([P, L], fp32, tag="y0")
                for ky in range(5):
                    src = bass.AP(xpad_d, i * 1296 + ky * 36,
                                  [[1, 5], [36, 32], [1, 32]])
                    nc.gpsimd.dma_start(y0[ky * 5:(ky + 1) * 5, :], src)
                nz = imgp.tile([P, L], fp32, tag="nz")
                nc.gpsimd.dma_start(nz[:], noise_d[:, i * L:(i + 1) * L])
                y2 = imgp.tile([P, L], fp32, tag="y2")
                nc.vector.tensor_add(y2[:], y0[:], nz[:])
                # permute locations to (py, px, a, b) so pooling partners sit at
                # the same position of 4 contiguous 256-blocks
                yp = imgp.tile([P, L], fp32, tag="yp")
                nc.vector.tensor_copy(
                    yp[:].rearrange("p (py px a b) -> p py px a b", py=2, px=2, a=16),
                    y2[:].rearrange("p (a py b px) -> p py px a b", py=2, b=16, px=2),
                )
                yr = imgp.tile([P, L], fp32r, tag="yr")
                nc.vector.tensor_copy(yr[:], yp[:])
                ysq = imgp.tile([P, L], fp32r, tag="ysq")
                nc.scalar.square(ysq[:], yp[:])

                for hf in range(2):
                    tp = psT.tile([OUT_CH, 128], fp32)
                    for k in range(4):
                        off = k * 256 + hf * 128
                        lhs = yr[:, off:off + 128]
                        lhs_sq = ysq[:, off:off + 128]
                        pA = psA.tile([128, 258], fp32)
                        nc.tensor.matmul(pA[:, 0:258], lhs, qr[:, 0:258],
                                         start=True, stop=False)
                        nc.tensor.matmul(pA[:, 256:258], lhs_sq, qr[:, 258:260],
                                         start=False, stop=True)
                        sc = smallp.tile([128, 2], fp32, tag="sc")
                        nc.vector.tensor_copy(sc[:], pA[:, 256:258])
                        tsq = workp.tile([128, 256], fp32, tag="tsq")
                        nc.scalar.square(tsq[:], pA[:, 0:256])
                        ssum = workp.tile([128, 64], fp32, tag="ssum")
                        nc.vector.tensor_reduce(
                            ssum[:],
                            tsq[:].rearrange("l (o c) -> l o c", c=4),
                            mybir.AxisListType.X,
                            mybir.AluOpType.add,
                        )
                        s2 = smallp.tile([128, 1], fp32, tag="s2")
                        nc.vector.tensor_mul(s2[:], sc[:, 0:1], sc[:, 0:1])
                        d = smallp.tile([128, 1], fp32, tag="d")
                        nc.vector.tensor_scalar_mul(d[:], sc[:, 1:2], 25.0)
                        nc.vector.tensor_sub(d[:], d[:], s2[:])
                        r24 = smallp.tile([128, 1], fp32, tag="r24")
                        nc.vector.reciprocal(r24[:], d[:])
                        nc.vector.tensor_scalar_mul(r24[:], r24[:], 24.0)
                        bia = smallp.tile([128, 1], fp32, tag="bia")
                        nc.vector.tensor_scalar(
                            bia[:], sc[:, 1:2], r24[:], -1.0,
                            mybir.AluOpType.mult, mybir.AluOpType.mult,
                        )
                        e = workp.tile([128, 64], fp32, tag="e")
                        nc.scalar.activation(
                            e[:], ssum[:], mybir.ActivationFunctionType.Exp,
                            bias=bia[:], scale=r24[:],
                        )
                        nc.tensor.matmul(tp[:], e[:], idt[:, 0:128],
                                         is_transpose=True,
                                         start=(k == 0), stop=(k == 3))
                    nc.vector.tensor_copy(
                        h1S_t[:, i * 256 + hf * 128:i * 256 + hf * 128 + 128], tp[:]
                    )

            s12_t = smallp.tile([OUT_CH, 2], fp32, tag="s12")
            nc.vector.tensor_reduce(
                s12_t[:, 0:1], h1S_t[:], mybir.AxisListType.X, mybir.AluOpType.add
            )
            scratch = outp.tile([OUT_CH, IMGS * 256], fp32)
            nc.scalar.activation(
                scratch[:], h1S_t[:], mybir.ActivationFunctionType.Square,
                accum_out=s12_t[:, 1:2],
            )
            nc.gpsimd.dma_start(h1S_d[:], h1S_t[:])
            nc.gpsimd.dma_start(s12_d[:], s12_t[:])
    nc.finalize()
    return nc


def _build_phase2():
    import concourse.bacc as bacc
    import concourse.bass as bass
    import concourse.mybir as mybir
    import concourse.tile as tile

    fp32 = mybir.dt.float32
    fp32r = mybir.dt.float32r
    AF = mybir.ActivationFunctionType
    AX = mybir.AxisListType
    OP = mybir.AluOpType
    nc = bacc.Bacc(None, target_bir_lowering=False)

    h1n_d = nc.declare_dram_parameter("h1n", [65, B * 256], fp32, isOutput=False)
    w2_d = nc.declare_dram_parameter("w2", [65, 9 * 128], fp32, isOutput=False)
    w3_d = nc.declare_dram_parameter("w3", [128, 9 * 128], fp32, isOutput=False)
    w4_d = nc.declare_dram_parameter("w4", [128, 9 * 128], fp32, isOutput=False)
    fc1_d = nc.declare_dram_parameter("fc1", [128, 128], fp32, isOutput=False)
    fc2_d = nc.declare_dram_parameter("fc2", [128, 10], fp32, isOutput=False)
    bn_d = nc.declare_dram_parameter("bng", [128, 8], fp32, isOutput=False)
    xm_d = nc.declare_dram_parameter("xm", [10, B], fp32, isOutput=True)
    xe_d = nc.declare_dram_parameter("xe", [128, B], fp32, isOutput=True)

    def bn_affine(tc, nc, pool, s1, s2, n, g, b, eps):
        """per-channel affine s,t from sums: s = g*rsqrt(var+eps), t = b - mu*s"""
        mu = pool.tile([128, 1], fp32, tag="bn_mu")
        nc.vector.tensor_scalar_mul(mu[:], s1, 1.0 / n)
        ve = pool.tile([128, 1], fp32, tag="bn_ve")
        nc.vector.tensor_scalar(ve[:], s2, 1.0 / n, eps, OP.mult, OP.add)
        msq = pool.tile([128, 1], fp32, tag="bn_msq")
        nc.vector.tensor_mul(msq[:], mu[:], mu[:])
        nc.vector.tensor_sub(ve[:], ve[:], msq[:])
        nc.scalar.sqrt(ve[:], ve[:])
        s = pool.tile([128, 1], fp32, tag="bn_s")
        nc.vector.reciprocal(s[:], ve[:])
        nc.vector.tensor_mul(s[:], s[:], g)
        t = pool.tile([128, 1], fp32, tag="bn_t")
        nc.vector.tensor_mul(t[:], mu[:], s[:])
        nc.vector.tensor_scalar(t[:], t[:], -1.0, None, OP.mult)
        nc.vector.tensor_add(t[:], t[:], b)
        return s, t

    with tile.TileContext(nc) as tc:
        with (
            tc.tile_pool(name="const", bufs=1) as constp,
            tc.tile_pool(name="acts", bufs=1) as actp,
            tc.tile_pool(name="stream", bufs=2) as strp,
            tc.tile_pool(name="work", bufs=1) as workp,
            tc.tile_pool(name="small", bufs=4) as smallp,
            tc.tile_pool(name="ps", bufs=4, space="PSUM") as psp,
        ):
            # ---- load + round weights
            def load_r(dram, shape):
                f = workp.tile(shape, fp32, tag="wload")
                nc.gpsimd.dma_start(f[:], dram[:])
                r = constp.tile(shape, fp32r)
                nc.vector.tensor_copy(r[:], f[:])
                return r

            w2 = load_r(w2_d, [65, 1152])
            w3 = load_r(w3_d, [128, 1152])
            w4 = load_r(w4_d, [128, 1152])
            fc1 = load_r(fc1_d, [128, 128])
            fc2 = load_r(fc2_d, [128, 10])
            bng = constp.tile([128, 8], fp32)
            nc.gpsimd.dma_start(bng[:], bn_d[:])

            # ---- conv2, streaming h1 in eighths of 8 imgs; stats fused into
            # the psum->sbuf copies via accum_out columns
            c2 = actp.tile([128, B * 256], fp32, tag="big")  # later reused
            s1c = smallp.tile([128, 32], fp32, tag="s1c")
            s2c = smallp.tile([128, 32], fp32, tag="s2c")
            sq_scr = actp.tile([128, 512], fp32, tag="sqscr")
            for e8 in range(8):
                h1f = strp.tile([65, 8 * 324], fp32, tag="h1f")
                nc.vector.memset(h1f[:], 0.0)
                for ii in range(8):
                    dst = bass.AP(h1f.tensor, ii * 324 + 19,
                                  [[h1f[:].ap[0][0], 65], [18, 16], [1, 16]])
                    off = (e8 * 8 + ii) * 256
                    nc.gpsimd.dma_start(dst, h1n_d[:, off:off + 256])
                h1r = strp.tile([65, 8 * 324], fp32r, tag="h1r")
                nc.vector.tensor_copy(h1r[:], h1f[:])
                for gi in range(4):  # 2-img groups within the eighth
                    g = e8 * 4 + gi
                    pc = psp.tile([128, 512], fp32, tag="pc")
                    for tap in range(9):
                        ky, kx = tap // 3, tap % 3
                        rhs = bass.AP(
                            h1r.tensor, gi * 2 * 324 + ky * 18 + kx,
                            [[h1r[:].ap[0][0], 65], [324, 2], [18, 16], [1, 16]],
                        )
                        nc.tensor.matmul(
                            pc[:], w2[:, tap * 128:(tap + 1) * 128], rhs,
                            start=(tap == 0), stop=(tap == 8),
                        )
                    nc.scalar.activation(
                        c2[:, g * 512:(g + 1) * 512], pc[:], AF.Copy,
                        accum_out=s1c[:, g:g + 1],
                    )
                    nc.scalar.activation(
                        sq_scr[:], pc[:], AF.Square, accum_out=s2c[:, g:g + 1],
                    )
            c2s1 = smallp.tile([128, 1], fp32, tag="c2s1")
            c2s2 = smallp.tile([128, 1], fp32, tag="c2s2")
            nc.vector.tensor_reduce(c2s1[:], s1c[:], AX.X, OP.add)
            nc.vector.tensor_reduce(c2s2[:], s2c[:], AX.X, OP.add)
            s2a, t2a = bn_affine(tc, nc, smallp, c2s1[:], c2s2[:], B * 256,
                                 bng[:, 0:1], bng[:, 1:2], BN_EPS)
            # bn2 + relu in place
            nc.scalar.activation(c2[:], c2[:], AF.Relu, bias=t2a[:], scale=s2a[:])
            # avgpool (sum; /4 folded into w3) -> h2p padded 10x10 interior 8x8
            h2p = actp.tile([128, B * 100], fp32r, tag="h2p")
            nc.vector.memset(h2p[:].bitcast(mybir.dt.uint32), 0)
            cs = actp.tile([128, B * 128], fp32, tag="mid")  # later reused
            h2v = c2[:].rearrange("c (i y x two) -> c i y x two", i=B, y=16, two=2)
            nc.vector.tensor_add(
                cs[:].rearrange("c (i y x) -> c i y x", i=B, y=16),
                h2v[:, :, :, :, 0], h2v[:, :, :, :, 1],
            )
            cv = cs[:].rearrange("c (i y two x) -> c i y two x", i=B, y=8, two=2)
            h2pi = bass.AP(
                h2p.tensor, 11, [[h2p[:].ap[0][0], 128], [100, B], [10, 8], [1, 8]]
            )
            nc.vector.tensor_add(h2pi, cv[:, :, :, 0, :], cv[:, :, :, 1, :])

            # ---- conv3: 8x8, groups of 4 imgs (N=256)
            c3 = actp.tile([128, B * 64], fp32, tag="c3")
            for g in range(B // 4):
                pc = psp.tile([128, 256], fp32, tag="pc")
                for tap in range(9):
                    ky, kx = tap // 3, tap % 3
                    rhs = bass.AP(
                        h2p.tensor, g * 4 * 100 + ky * 10 + kx,
                        [[h2p[:].ap[0][0], 128], [100, 4], [10, 8], [1, 8]],
                    )
                    nc.tensor.matmul(
                        pc[:], w3[:, tap * 128:(tap + 1) * 128], rhs,
                        start=(tap == 0), stop=(tap == 8),
                    )
                nc.scalar.activation(
                    c3[:, g * 256:(g + 1) * 256], pc[:], AF.Copy,
                    accum_out=s1c[:, g:g + 1],
                )
                nc.scalar.activation(
                    sq_scr[:, 0:256], pc[:], AF.Square, accum_out=s2c[:, g:g + 1],
                )
            c3s1 = smallp.tile([128, 1], fp32, tag="c3s1")
            c3s2 = smallp.tile([128, 1], fp32, tag="c3s2")
            nc.vector.tensor_reduce(c3s1[:], s1c[:, 0:16], AX.X, OP.add)
            nc.vector.tensor_reduce(c3s2[:], s2c[:, 0:16], AX.X, OP.add)
            s3a, t3a = bn_affine(tc, nc, smallp, c3s1[:], c3s2[:], B * 64,
                                 bng[:, 2:3], bng[:, 3:4], BN_EPS)
            # bn3+relu into zero-padded 10x10 (relu>=0 so 0-pad == -inf pad)
            h3m = actp.tile([128, B * 100], fp32, tag="big")
            nc.vector.memset(h3m[:], 0.0)
            h3mi = bass.AP(
                h3m.tensor, 11, [[h3m[:].ap[0][0], 128], [100, B], [10, 8], [1, 8]]
            )
            nc.scalar.activation(
                h3mi, c3[:].rearrange("c (i y x) -> c i y x", i=B, y=8),
                AF.Relu, bias=t3a[:], scale=s3a[:],
            )
            # maxpool k2 s2 pad1 -> 5x5
            m1 = actp.tile([128, B * 50], fp32, tag="mid")
            h3v = h3m[:].rearrange("c (i y x two) -> c i y x two", i=B, y=10, two=2)
            nc.vector.tensor_max(
                m1[:].rearrange("c (i y x) -> c i y x", i=B, y=10),
                h3v[:, :, :, :, 0], h3v[:, :, :, :, 1],
            )
            h4p = actp.tile([128, B * 49 + 16], fp32r, tag="c3")
            nc.vector.memset(h4p[:].bitcast(mybir.dt.uint32), 0)
            m1v = m1[:].rearrange("c (i y two x) -> c i y two x", i=B, y=5, two=2)
            h4pi = bass.AP(
                h4p.tensor, 8, [[h4p[:].ap[0][0], 128], [49, B], [7, 5], [1, 5]]
            )
            nc.vector.tensor_max(h4pi, m1v[:, :, :, 0, :], m1v[:, :, :, 1, :])

            # ---- conv4: 5x5 (pad 7x7), groups of 8 imgs, 6x6 over-read (N=288)
            c4 = actp.tile([128, B * 36], fp32, tag="mid")
            for g in range(B // 8):
                pc = psp.tile([128, 288], fp32, tag="pc")
                for tap in range(9):
                    ky, kx = tap // 3, tap % 3
                    rhs = bass.AP(
                        h4p.tensor, g * 8 * 49 + ky * 7 + kx,
                        [[h4p[:].ap[0][0], 128], [49, 8], [7, 6], [1, 6]],
                    )
                    nc.tensor.matmul(
                        pc[:], w4[:, tap * 128:(tap + 1) * 128], rhs,
                        start=(tap == 0), stop=(tap == 8),
                    )
                nc.scalar.activation(c4[:, g * 288:(g + 1) * 288], pc[:], AF.Copy)
            # stats over the 5x5 valid region only
            c4v = c4[:].rearrange("c (i y x) -> c i y x", i=B, y=6)[:, :, 0:5, 0:5]
            c4s1 = smallp.tile([128, 1], fp32, tag="c4s1")
            c4s2 = smallp.tile([128, 1], fp32, tag="c4s2")
            nc.vector.tensor_reduce(c4s1[:], c4v, AX.XYZ, OP.add)
            scr4 = actp.tile([128, B * 25], fp32, tag="big")
            nc.scalar.activation(
                scr4[:].rearrange("c (i y x) -> c i y x", i=B, y=5), c4v,
                AF.Square, accum_out=c4s2[:],
            )
            s4a, t4a = bn_affine(tc, nc, smallp, c4s1[:], c4s2[:], B * 25,
                                 bng[:, 4:5], bng[:, 5:6], BN_EPS)
            h4 = actp.tile([128, B * 25], fp32, tag="h4")
            nc.scalar.activation(
                h4[:].rearrange("c (i y x) -> c i y x", i=B, y=5), c4v,
                AF.Relu, bias=t4a[:], scale=s4a[:],
            )
            # maxpool k4: max over rows 0..3 x cols 0..3 of the 5x5
            xe_t = actp.tile([128, B], fp32, tag="xe")
            h4g = h4[:].rearrange("c (i y x) -> c i y x", i=B, y=5)[:, :, 0:4, 0:4]
            nc.vector.tensor_reduce(xe_t[:], h4g, AX.XY, OP.max)
            nc.gpsimd.dma_start(xe_d[:], xe_t[:])
            xer = actp.tile([128, B], fp32r, tag="xer")
            nc.vector.tensor_copy(xer[:], xe_t[:])

            # ---- fc1 + bnfc + relu + fc2
            pf = psp.tile([128, B], fp32, tag="pc")
            nc.tensor.matmul(pf[:], fc1[:], xer[:], start=True, stop=True)
            f1 = workp.tile([128, B], fp32, tag="f1")
            fs1 = smallp.tile([128, 1], fp32, tag="fs1")
            fs2 = smallp.tile([128, 1], fp32, tag="fs2")
            nc.scalar.activation(f1[:], pf[:], AF.Copy, accum_out=fs1[:])
            scf = workp.tile([128, B], fp32, tag="scf")
            nc.scalar.activation(scf[:], f1[:], AF.Square, accum_out=fs2[:])
            sfa, tfa = bn_affine(tc, nc, smallp, fs1[:], fs2[:], B,
                                 bng[:, 6:7], bng[:, 7:8], BN_EPS)
            xfc = workp.tile([128, B], fp32r, tag="xfc")
            nc.scalar.activation(xfc[:], f1[:], AF.Relu, bias=tfa[:], scale=sfa[:])
            pm = psp.tile([10, B], fp32, tag="pc")
            nc.tensor.matmul(pm[:], fc2[:], xfc[:], start=True, stop=True)
            xm_t = workp.tile([10, B], fp32, tag="xm")
            nc.scalar.copy(xm_t[:], pm[:])
            nc.gpsimd.dma_start(xm_d[:], xm_t[:])
    nc.finalize()
    return nc


def kernel(x, gmu_w, conv2_w, conv2_b, conv3_w, conv3_b, conv4_w, conv4_b,
           fc1_w, fc1_b, fc2_w, fc2_b,
           bn1_g, bn1_b, bn2_g, bn2_b, bn3_g, bn3_b, bn4_g, bn4_b,
           bnfc_g, bnfc_b):
    from concourse.bass_utils import run_bass_kernel_spmd

    x = np.asarray(x, np.float32)
    qmat = _host_constants(np.asarray(gmu_w, np.float32))
    if "noise" not in _cache:
        _cache["noise"] = _noise_shards()
    noise_sh = _cache["noise"]

    # ---- phase 1
    if "nc1" not in _cache:
        _cache["nc1"] = _build_phase1()
    nc1 = _cache["nc1"]
    xp = np.zeros((B, 36, 36), np.float32)
    xp[:, 2:34, 2:34] = x[:, 0]
    ident = np.eye(128, dtype=np.float32)
    in_maps1 = [
        {
            "xpad": xp[c * IMGS:(c + 1) * IMGS].reshape(-1),
            "noise": noise_sh[c],
            "qmat": qmat,
            "ident": ident,
        }
        for c in range(N_CORES)
    ]
    res1 = run_bass_kernel_spmd(nc1, in_maps1, list(range(N_CORES))).results

    # ---- host: exact global BN1 stats + weight folding
    h1S = np.concatenate(
        [res1[c]["h1S"].reshape(OUT_CH, IMGS, 256) for c in range(N_CORES)], axis=1
    )  # (64, B, 256) sum-pooled exp values, spatial (y,x) 16x16
    S1 = sum(res1[c]["s12"][:, 0] for c in range(N_CORES))
    S2 = sum(res1[c]["s12"][:, 1] for c in range(N_CORES))
    nS = B * 256
    muP = S1 / nS
    varP = S2 / nS - muP * muP
    s1f = np.asarray(bn1_g) / np.sqrt(varP + 16.0 * BN_EPS / (C1 * C1))
    t1f = np.asarray(bn1_b) - muP * s1f

    # conv2 weights: fold s1 into ic columns; bias channel = conv of t1 map
    w2 = np.asarray(conv2_w, np.float32)  # (128, 64, 3, 3)
    w2f = w2 * s1f[None, :, None, None]
    tmap = np.einsum("oikl,i->okl", w2, t1f)  # (128, 3, 3)
    w2aug = np.zeros((65, 9 * 128), np.float32)
    for tap in range(9):
        ky, kx = tap // 3, tap % 3
        w2aug[:64, tap * 128:(tap + 1) * 128] = w2f[:, :, ky, kx].T
        w2aug[64, tap * 128:(tap + 1) * 128] = tmap[:, ky, kx]
    # compact h1: rows 0-63 = h1S, row 64 = ones channel (padding done on device)
    h1n = np.ones((65, B * 256), np.float32)
    h1n[:64] = h1S.reshape(OUT_CH, B * 256)

    def taps(wc):  # (oc, ic, 3, 3) -> (ic, 9*oc)
        out = np.zeros((wc.shape[1], 9 * wc.shape[0]), np.float32)
        for tap in range(9):
            ky, kx = tap // 3, tap % 3
            out[:, tap * wc.shape[0]:(tap + 1) * wc.shape[0]] = wc[:, :, ky, kx].T
        return out

    w3t = taps(np.asarray(conv3_w, np.float32) * 0.25)
    w4t = taps(np.asarray(conv4_w, np.float32))
    bng = np.stack(
        [np.asarray(v, np.float32) for v in
         (bn2_g, bn2_b, bn3_g, bn3_b, bn4_g, bn4_b, bnfc_g, bnfc_b)], axis=1
    )  # (128, 8)

    if "nc2" not in _cache:
        _cache["nc2"] = _build_phase2()
    nc2 = _cache["nc2"]
    im2 = {
        "h1n": h1n,
        "w2": w2aug,
        "w3": w3t,
        "w4": w4t,
        "fc1": np.asarray(fc1_w, np.float32).reshape(128, 128).T.copy(),
        "fc2": np.asarray(fc2_w, np.float32).reshape(10, 128).T.copy(),
        "bng": bng,
    }
    res2 = run_bass_kernel_spmd(nc2, [im2] * N_CORES, list(range(N_CORES))).results

    xm = res2[0]["xm"].T.copy() + np.asarray(fc2_b, np.float32)[None, :]  # (B, 10)
    xe = res2[0]["xe"].T.reshape(B, 128, 1, 1).copy()
    return xm, xe


# revision 17
# speedup vs baseline: 5583.4539x; 5236.2511x over previous
"""Trainium2 Bass kernel for nn_Net_vanilla_CNN (GMU local-regression + dense CNN).

Strategy (8 NeuronCores):
  Phase 1 (batch-sharded, 8 imgs/core): im2col + noise -> t = Q^T y matmul
    (fp32r) -> err -> exp activation -> PE-transpose with PSUM accumulation
    (does the 2x2 sum-pool for free) -> h1S (sum-pooled pre-BN1 features)
    + per-core partial BN1 sums.
  Host: exact global BN1 stats; folds BN1 affine into conv2 weights
    (scale into weights, bias via an appended ones-channel).
  Phase 2 (replicated, full batch on every core): conv2/3/4 + BN2-4 +
    pools + fc1 + BNfc + fc2, with exact global BN stats computed on-device
    (full batch is local, so no cross-device communication anywhere).

BN eps/affine folding identities used (all exact):
  - conv biases feeding a BN cancel (BN subtracts the mean) -> dropped.
  - a = c1*exp(-err)+c0 then avgpool then BN1  ==  BN with adjusted eps on
    sum-pooled exp(-err):  s = g1*rsqrt(var_P + 16*eps/c1^2), t = b1 - mu_P*s.
  - avgpool /4 before conv3 folded into conv3 weights.
  - maxpool pad(-inf) on relu outputs == pad(0).
"""
import math
import numpy as np

B, C_IN, H, W = 64, 1, 32, 32
K, PAD, NS, OUT_CH = 5, 2, 3, 64
P = C_IN * K * K  # 25
L = H * W  # 1024
EPS_NOISE = 1e-4
BN_EPS = 1e-5
N_CORES = 8
IMGS = B // N_CORES  # 8
C1 = 1.0 / (1.0 - math.exp(-1.0))  # exp activation scale

_cache = {}


def _host_constants(gmu_w):
    """Q matrix (25 x 257): stacked per-o orthonormal bases + ones column."""
    Xm = gmu_w.reshape(OUT_CH, P, NS).astype(np.float64)
    Xm = np.concatenate([np.ones((OUT_CH, P, 1)), Xm], axis=2)  # (64,25,4)
    cov = np.einsum("opc,opd->ocd", Xm, Xm)
    Lc = np.linalg.cholesky(cov)  # (64,4,4)
    # Q_o = Xm_o @ inv(Lc_o).T  -> orthonormal columns
    Q = np.einsum("opd,ocd->opc", Xm, np.linalg.inv(Lc))  # (64,25,4)
    qmat = np.zeros((P, 260), np.float32)
    qmat[:, :256] = Q.transpose(1, 0, 2).reshape(P, 256)
    qmat[:, 256] = 1.0  # sum column (col 257 stays 0)
    qmat[:, 259] = 1.0  # cols 258:260 = [0 | 1]: ysq-matmul rhs -> [0, sumsq]
    return qmat


def _noise_shards():
    import jax

    with jax.default_device(jax.devices("cpu")[0]):
        noise = np.asarray(
            jax.random.normal(jax.random.key(42), (B, P, L), "float32")
        ) * np.float32(EPS_NOISE)
    # per-core (25, IMGS*1024) layout [p, img*L + l]
    return [
        noise[c * IMGS:(c + 1) * IMGS].transpose(1, 0, 2).reshape(P, IMGS * L)
        for c in range(N_CORES)
    ]


def _build_phase1():
    import concourse.bacc as bacc
    import concourse.bass as bass
    import concourse.mybir as mybir
    import concourse.tile as tile

    fp32 = mybir.dt.float32
    fp32r = mybir.dt.float32r
    nc = bacc.Bacc(None, target_bir_lowering=False)
    xpad_d = nc.declare_dram_parameter("xpad", [IMGS * 1296], fp32, isOutput=False)
    noise_d = nc.declare_dram_parameter("noise", [P, IMGS * L], fp32, isOutput=False)
    qmat_d = nc.declare_dram_parameter("qmat", [P, 260], fp32, isOutput=False)
    ident_d = nc.declare_dram_parameter("ident", [128, 128], fp32, isOutput=False)
    h1S_d = nc.declare_dram_parameter("h1S", [OUT_CH, IMGS * 256], fp32, isOutput=True)
    s12_d = nc.declare_dram_parameter("s12", [OUT_CH, 2], fp32, isOutput=True)

    with tile.TileContext(nc) as tc:
        with (
            tc.tile_pool(name="const", bufs=1) as constp,
            tc.tile_pool(name="imgs", bufs=3) as imgp,
            tc.tile_pool(name="work", bufs=6) as workp,
            tc.tile_pool(name="small", bufs=16) as smallp,
            tc.tile_pool(name="out", bufs=1) as outp,
            tc.tile_pool(name="psA", bufs=5, space="PSUM") as psA,
            tc.tile_pool(name="psT", bufs=2, space="PSUM") as psT,
        ):
            qf = constp.tile([P, 260], fp32)
            nc.gpsimd.dma_start(qf[:], qmat_d[:])
            qr = constp.tile([P, 260], fp32r)
            nc.vector.tensor_copy(qr[:], qf[:])
            idt = constp.tile([128, 128], fp32)
            nc.gpsimd.dma_start(idt[:], ident_d[:])
            h1S_t = outp.tile([OUT_CH, IMGS * 256], fp32)

            for i in range(IMGS):
                y0 = imgp.tile([P, L], fp32, tag="y0")
                for ky in range(5):
                    src = bass.AP(xpad_d, i * 1296 + ky * 36,
                                  [[1, 5], [36, 32], [1, 32]])
                    nc.gpsimd.dma_start(y0[ky * 5:(ky + 1) * 5, :], src)
                nz = imgp.tile([P, L], fp32, tag="nz")
                nc.gpsimd.dma_start(nz[:], noise_d[:, i * L:(i + 1) * L])
                y2 = imgp.tile([P, L], fp32, tag="y2")
                nc.vector.tensor_add(y2[:], y0[:], nz[:])
                # permute locations to (py, px, a, b) so pooling partners sit at
                # the same position of 4 contiguous 256-blocks
                yp = imgp.tile([P, L], fp32, tag="yp")
                nc.vector.tensor_copy(
                    yp[:].rearrange("p (py px a b) -> p py px a b", py=2, px=2, a=16),
                    y2[:].rearrange("p (a py b px) -> p py px a b", py=2, b=16, px=2),
                )
                yr = imgp.tile([P, L], fp32r, tag="yr")
                nc.vector.tensor_copy(yr[:], yp[:])
                ysq = imgp.tile([P, L], fp32r, tag="ysq")
                nc.scalar.square(ysq[:], yp[:])

                for hf in range(2):
                    tp = psT.tile([OUT_CH, 128], fp32)
                    for k in range(4):
                        off = k * 256 + hf * 128
                        lhs = yr[:, off:off + 128]
                        lhs_sq = ysq[:, off:off + 128]
                        pA = psA.tile([128, 258], fp32)
                        nc.tensor.matmul(pA[:, 0:258], lhs, qr[:, 0:258],
                                         start=True, stop=False)
                        nc.tensor.matmul(pA[:, 256:258], lhs_sq, qr[:, 258:260],
                                         start=False, stop=True)
                        sc = smallp.tile([128, 2], fp32, tag="sc")
                        nc.vector.tensor_copy(sc[:], pA[:, 256:258])
                        tsq = workp.tile([128, 256], fp32, tag="tsq")
                        nc.scalar.square(tsq[:], pA[:, 0:256])
                        ssum = workp.tile([128, 64], fp32, tag="ssum")
                        nc.vector.tensor_reduce(
                            ssum[:],
                            tsq[:].rearrange("l (o c) -> l o c", c=4),
                            mybir.AxisListType.X,
                            mybir.AluOpType.add,
                        )
                        s2 = smallp.tile([128, 1], fp32, tag="s2")
                        nc.vector.tensor_mul(s2[:], sc[:, 0:1], sc[:, 0:1])
                        d = smallp.tile([128, 1], fp32, tag="d")
                        nc.vector.tensor_scalar_mul(d[:], sc[:, 1:2], 25.0)
                        nc.vector.tensor_sub(d[:], d[:], s2[:])
                        r24 = smallp.tile([128, 1], fp32, tag="r24")
                        nc.vector.reciprocal(r24[:], d[:])
                        nc.vector.tensor_scalar_mul(r24[:], r24[:], 24.0)
                        bia = smallp.tile([128, 1], fp32, tag="bia")
                        nc.vector.tensor_scalar(
                            bia[:], sc[:, 1:2], r24[:], -1.0,
                            mybir.AluOpType.mult, mybir.AluOpType.mult,
                        )
                        e = workp.tile([128, 64], fp32, tag="e")
                        nc.scalar.activation(
                            e[:], ssum[:], mybir.ActivationFunctionType.Exp,
                            bias=bia[:], scale=r24[:],
                        )
                        nc.tensor.matmul(tp[:], e[:], idt[:, 0:128],
                                         is_transpose=True,
                                         start=(k == 0), stop=(k == 3))
                    nc.vector.tensor_copy(
                        h1S_t[:, i * 256 + hf * 128:i * 256 + hf * 128 + 128], tp[:]
                    )

            s12_t = smallp.tile([OUT_CH, 2], fp32, tag="s12")
            nc.vector.tensor_reduce(
                s12_t[:, 0:1], h1S_t[:], mybir.AxisListType.X, mybir.AluOpType.add
            )
            scratch = outp.tile([OUT_CH, IMGS * 256], fp32)
            nc.scalar.activation(
                scratch[:], h1S_t[:], mybir.ActivationFunctionType.Square,
                accum_out=s12_t[:, 1:2],
            )
            nc.gpsimd.dma_start(h1S_d[:], h1S_t[:])
            nc.gpsimd.dma_start(s12_d[:], s12_t[:])
    nc.finalize()
    return nc


def _build_phase2():
    import concourse.bacc as bacc
    import concourse.bass as bass
    import concourse.mybir as mybir
    import concourse.tile as tile

    fp32 = mybir.dt.float32
    fp32r = mybir.dt.float32r
    AF = mybir.ActivationFunctionType
    AX = mybir.AxisListType
    OP = mybir.AluOpType
    nc = bacc.Bacc(None, target_bir_lowering=False)

    h1n_d = nc.declare_dram_parameter("h1n", [65, B * 256], fp32, isOutput=False)
    w2_d = nc.declare_dram_parameter("w2", [65, 9 * 128], fp32, isOutput=False)
    w3_d = nc.declare_dram_parameter("w3", [128, 9 * 128], fp32, isOutput=False)
    w4_d = nc.declare_dram_parameter("w4", [128, 9 * 128], fp32, isOutput=False)
    fc1_d = nc.declare_dram_parameter("fc1", [128, 128], fp32, isOutput=False)
    fc2_d = nc.declare_dram_parameter("fc2", [128, 10], fp32, isOutput=False)
    bn_d = nc.declare_dram_parameter("bng", [128, 8], fp32, isOutput=False)
    xm_d = nc.declare_dram_parameter("xm", [10, B], fp32, isOutput=True)
    xe_d = nc.declare_dram_parameter("xe", [128, B], fp32, isOutput=True)

    def bn_affine(tc, nc, pool, s1, s2, n, g, b, eps):
        """per-channel affine s,t from sums: s = g*rsqrt(var+eps), t = b - mu*s"""
        mu = pool.tile([128, 1], fp32, tag="bn_mu")
        nc.vector.tensor_scalar_mul(mu[:], s1, 1.0 / n)
        ve = pool.tile([128, 1], fp32, tag="bn_ve")
        nc.vector.tensor_scalar(ve[:], s2, 1.0 / n, eps, OP.mult, OP.add)
        msq = pool.tile([128, 1], fp32, tag="bn_msq")
        nc.vector.tensor_mul(msq[:], mu[:], mu[:])
        nc.vector.tensor_sub(ve[:], ve[:], msq[:])
        nc.scalar.sqrt(ve[:], ve[:])
        s = pool.tile([128, 1], fp32, tag="bn_s")
        nc.vector.reciprocal(s[:], ve[:])
        nc.vector.tensor_mul(s[:], s[:], g)
        t = pool.tile([128, 1], fp32, tag="bn_t")
        nc.vector.tensor_mul(t[:], mu[:], s[:])
        nc.vector.tensor_scalar(t[:], t[:], -1.0, None, OP.mult)
        nc.vector.tensor_add(t[:], t[:], b)
        return s, t

    with tile.TileContext(nc) as tc:
        with (
            tc.tile_pool(name="const", bufs=1) as constp,
            tc.tile_pool(name="acts", bufs=1) as actp,
            tc.tile_pool(name="stream", bufs=2) as strp,
            tc.tile_pool(name="work", bufs=1) as workp,
            tc.tile_pool(name="small", bufs=4) as smallp,
            tc.tile_pool(name="ps", bufs=4, space="PSUM") as psp,
        ):
            # ---- load + round weights
            def load_r(dram, shape):
                f = workp.tile(shape, fp32, tag="wload")
                nc.gpsimd.dma_start(f[:], dram[:])
                r = constp.tile(shape, fp32r)
                nc.vector.tensor_copy(r[:], f[:])
                return r

            w2 = load_r(w2_d, [65, 1152])
            w3 = load_r(w3_d, [128, 1152])
            w4 = load_r(w4_d, [128, 1152])
            fc1 = load_r(fc1_d, [128, 128])
            fc2 = load_r(fc2_d, [128, 10])
            bng = constp.tile([128, 8], fp32)
            nc.gpsimd.dma_start(bng[:], bn_d[:])

            # ---- conv2, streaming h1 in eighths of 8 imgs; stats fused into
            # the psum->sbuf copies via accum_out columns
            c2 = actp.tile([128, B * 256], fp32, tag="big")  # later reused
            s1c = smallp.tile([128, 32], fp32, tag="s1c")
            s2c = smallp.tile([128, 32], fp32, tag="s2c")
            sq_scr = actp.tile([128, 512], fp32, tag="sqscr")
            for e8 in range(8):
                h1f = strp.tile([65, 8 * 324], fp32, tag="h1f")
                nc.vector.memset(h1f[:], 0.0)
                for ii in range(8):
                    dst = bass.AP(h1f.tensor, ii * 324 + 19,
                                  [[h1f[:].ap[0][0], 65], [18, 16], [1, 16]])
                    off = (e8 * 8 + ii) * 256
                    nc.gpsimd.dma_start(dst, h1n_d[:, off:off + 256])
                h1r = strp.tile([65, 8 * 324], fp32r, tag="h1r")
                nc.vector.tensor_copy(h1r[:], h1f[:])
                for gi in range(4):  # 2-img groups within the eighth
                    g = e8 * 4 + gi
                    pc = psp.tile([128, 512], fp32, tag="pc")
                    for tap in range(9):
                        ky, kx = tap // 3, tap % 3
                        rhs = bass.AP(
                            h1r.tensor, gi * 2 * 324 + ky * 18 + kx,
                            [[h1r[:].ap[0][0], 65], [324, 2], [18, 16], [1, 16]],
                        )
                        nc.tensor.matmul(
                            pc[:], w2[:, tap * 128:(tap + 1) * 128], rhs,
                            start=(tap == 0), stop=(tap == 8),
                        )
                    nc.scalar.activation(
                        c2[:, g * 512:(g + 1) * 512], pc[:], AF.Copy,
                        accum_out=s1c[:, g:g + 1],
                    )
                    nc.scalar.activation(
                        sq_scr[:], pc[:], AF.Square, accum_out=s2c[:, g:g + 1],
                    )
            c2s1 = smallp.tile([128, 1], fp32, tag="c2s1")
            c2s2 = smallp.tile([128, 1], fp32, tag="c2s2")
            nc.vector.tensor_reduce(c2s1[:], s1c[:], AX.X, OP.add)
            nc.vector.tensor_reduce(c2s2[:], s2c[:], AX.X, OP.add)
            s2a, t2a = bn_affine(tc, nc, smallp, c2s1[:], c2s2[:], B * 256,
                                 bng[:, 0:1], bng[:, 1:2], BN_EPS)
            # bn2 + relu in place
            nc.scalar.activation(c2[:], c2[:], AF.Relu, bias=t2a[:], scale=s2a[:])
            # avgpool (sum; /4 folded into w3) -> h2p padded 10x10 interior 8x8
            h2p = actp.tile([128, B * 100], fp32r, tag="h2p")
            nc.vector.memset(h2p[:].bitcast(mybir.dt.uint32), 0)
            cs = actp.tile([128, B * 128], fp32, tag="mid")  # later reused
            h2v = c2[:].rearrange("c (i y x two) -> c i y x two", i=B, y=16, two=2)
            nc.vector.tensor_add(
                cs[:].rearrange("c (i y x) -> c i y x", i=B, y=16),
                h2v[:, :, :, :, 0], h2v[:, :, :, :, 1],
            )
            cv = cs[:].rearrange("c (i y two x) -> c i y two x", i=B, y=8, two=2)
            h2pi = bass.AP(
                h2p.tensor, 11, [[h2p[:].ap[0][0], 128], [100, B], [10, 8], [1, 8]]
            )
            nc.vector.tensor_add(h2pi, cv[:, :, :, 0, :], cv[:, :, :, 1, :])

            # ---- conv3: 8x8, groups of 4 imgs (N=256)
            c3 = actp.tile([128, B * 64], fp32, tag="c3")
            for g in range(B // 4):
                pc = psp.tile([128, 256], fp32, tag="pc")
                for tap in range(9):
                    ky, kx = tap // 3, tap % 3
                    rhs = bass.AP(
                        h2p.tensor, g * 4 * 100 + ky * 10 + kx,
                        [[h2p[:].ap[0][0], 128], [100, 4], [10, 8], [1, 8]],
                    )
                    nc.tensor.matmul(
                        pc[:], w3[:, tap * 128:(tap + 1) * 128], rhs,
                        start=(tap == 0), stop=(tap == 8),
                    )
                nc.scalar.activation(
                    c3[:, g * 256:(g + 1) * 256], pc[:], AF.Copy,
                    accum_out=s1c[:, g:g + 1],
                )
                nc.scalar.activation(
                    sq_scr[:, 0:256], pc[:], AF.Square, accum_out=s2c[:, g:g + 1],
                )
            c3s1 = smallp.tile([128, 1], fp32, tag="c3s1")
            c3s2 = smallp.tile([128, 1], fp32, tag="c3s2")
            nc.vector.tensor_reduce(c3s1[:], s1c[:, 0:16], AX.X, OP.add)
            nc.vector.tensor_reduce(c3s2[:], s2c[:, 0:16], AX.X, OP.add)
            s3a, t3a = bn_affine(tc, nc, smallp, c3s1[:], c3s2[:], B * 64,
                                 bng[:, 2:3], bng[:, 3:4], BN_EPS)
            # bn3+relu into zero-padded 10x10 (relu>=0 so 0-pad == -inf pad)
            h3m = actp.tile([128, B * 100], fp32, tag="big")
            nc.vector.memset(h3m[:], 0.0)
            h3mi = bass.AP(
                h3m.tensor, 11, [[h3m[:].ap[0][0], 128], [100, B], [10, 8], [1, 8]]
            )
            nc.scalar.activation(
                h3mi, c3[:].rearrange("c (i y x) -> c i y x", i=B, y=8),
                AF.Relu, bias=t3a[:], scale=s3a[:],
            )
            # maxpool k2 s2 pad1 -> 5x5
            m1 = actp.tile([128, B * 50], fp32, tag="mid")
            h3v = h3m[:].rearrange("c (i y x two) -> c i y x two", i=B, y=10, two=2)
            nc.vector.tensor_max(
                m1[:].rearrange("c (i y x) -> c i y x", i=B, y=10),
                h3v[:, :, :, :, 0], h3v[:, :, :, :, 1],
            )
            h4p = actp.tile([128, B * 49 + 16], fp32r, tag="c3")
            nc.vector.memset(h4p[:].bitcast(mybir.dt.uint32), 0)
            m1v = m1[:].rearrange("c (i y two x) -> c i y two x", i=B, y=5, two=2)
            h4pi = bass.AP(
                h4p.tensor, 8, [[h4p[:].ap[0][0], 128], [49, B], [7, 5], [1, 5]]
            )
            nc.vector.tensor_max(h4pi, m1v[:, :, :, 0, :], m1v[:, :, :, 1, :])

            # ---- conv4: 5x5 (pad 7x7), groups of 8 imgs, 6x6 over-read (N=288)
            c4 = actp.tile([128, B * 36], fp32, tag="mid")
            for g in range(B // 8):
                pc = psp.tile([128, 288], fp32, tag="pc")
                for tap in range(9):
                    ky, kx = tap // 3, tap % 3
                    rhs = bass.AP(
                        h4p.tensor, g * 8 * 49 + ky * 7 + kx,
                        [[h4p[:].ap[0][0], 128], [49, 8], [7, 6], [1, 6]],
                    )
                    nc.tensor.matmul(
                        pc[:], w4[:, tap * 128:(tap + 1) * 128], rhs,
                        start=(tap == 0), stop=(tap == 8),
                    )
                nc.scalar.activation(c4[:, g * 288:(g + 1) * 288], pc[:], AF.Copy)
            # stats over the 5x5 valid region only
            c4v = c4[:].rearrange("c (i y x) -> c i y x", i=B, y=6)[:, :, 0:5, 0:5]
            c4s1 = smallp.tile([128, 1], fp32, tag="c4s1")
            c4s2 = smallp.tile([128, 1], fp32, tag="c4s2")
            nc.vector.tensor_reduce(c4s1[:], c4v, AX.XYZ, OP.add)
            scr4 = actp.tile([128, B * 25], fp32, tag="big")
            nc.scalar.activation(
                scr4[:].rearrange("c (i y x) -> c i y x", i=B, y=5), c4v,
                AF.Square, accum_out=c4s2[:],
            )
            s4a, t4a = bn_affine(tc, nc, smallp, c4s1[:], c4s2[:], B * 25,
                                 bng[:, 4:5], bng[:, 5:6], BN_EPS)
            h4 = actp.tile([128, B * 25], fp32, tag="h4")
            nc.scalar.activation(
                h4[:].rearrange("c (i y x) -> c i y x", i=B, y=5), c4v,
                AF.Relu, bias=t4a[:], scale=s4a[:],
            )
            # maxpool k4: max over rows 0..3 x cols 0..3 of the 5x5
            xe_t = actp.tile([128, B], fp32, tag="xe")
            h4g = h4[:].rearrange("c (i y x) -> c i y x", i=B, y=5)[:, :, 0:4, 0:4]
            nc.vector.tensor_reduce(xe_t[:], h4g, AX.XY, OP.max)
            nc.gpsimd.dma_start(xe_d[:], xe_t[:])
            xer = actp.tile([128, B], fp32r, tag="xer")
            nc.vector.tensor_copy(xer[:], xe_t[:])

            # ---- fc1 + bnfc + relu + fc2
            pf = psp.tile([128, B], fp32, tag="pc")
            nc.tensor.matmul(pf[:], fc1[:], xer[:], start=True, stop=True)
            f1 = workp.tile([128, B], fp32, tag="f1")
            fs1 = smallp.tile([128, 1], fp32, tag="fs1")
            fs2 = smallp.tile([128, 1], fp32, tag="fs2")
            nc.scalar.activation(f1[:], pf[:], AF.Copy, accum_out=fs1[:])
            scf = workp.tile([128, B], fp32, tag="scf")
            nc.scalar.activation(scf[:], f1[:], AF.Square, accum_out=fs2[:])
            sfa, tfa = bn_affine(tc, nc, smallp, fs1[:], fs2[:], B,
                                 bng[:, 6:7], bng[:, 7:8], BN_EPS)
            xfc = workp.tile([128, B], fp32r, tag="xfc")
            nc.scalar.activation(xfc[:], f1[:], AF.Relu, bias=tfa[:], scale=sfa[:])
            pm = psp.tile([10, B], fp32, tag="pc")
            nc.tensor.matmul(pm[:], fc2[:], xfc[:], start=True, stop=True)
            xm_t = workp.tile([10, B], fp32, tag="xm")
            nc.scalar.copy(xm_t[:], pm[:])
            nc.gpsimd.dma_start(xm_d[:], xm_t[:])
    nc.finalize()
    return nc


def kernel(x, gmu_w, conv2_w, conv2_b, conv3_w, conv3_b, conv4_w, conv4_b,
           fc1_w, fc1_b, fc2_w, fc2_b,
           bn1_g, bn1_b, bn2_g, bn2_b, bn3_g, bn3_b, bn4_g, bn4_b,
           bnfc_g, bnfc_b):
    from concourse.bass_utils import run_bass_kernel_spmd

    x = np.asarray(x, np.float32)
    qmat = _host_constants(np.asarray(gmu_w, np.float32))
    if "noise" not in _cache:
        _cache["noise"] = _noise_shards()
    noise_sh = _cache["noise"]

    # ---- phase 1
    if "nc1" not in _cache:
        _cache["nc1"] = _build_phase1()
    nc1 = _cache["nc1"]
    xp = np.zeros((B, 36, 36), np.float32)
    xp[:, 2:34, 2:34] = x[:, 0]
    ident = np.eye(128, dtype=np.float32)
    in_maps1 = [
        {
            "xpad": xp[c * IMGS:(c + 1) * IMGS].reshape(-1),
            "noise": noise_sh[c],
            "qmat": qmat,
            "ident": ident,
        }
        for c in range(N_CORES)
    ]
    res1 = run_bass_kernel_spmd(nc1, in_maps1, list(range(N_CORES))).results

    # ---- host: exact global BN1 stats + weight folding
    h1S = np.concatenate(
        [res1[c]["h1S"].reshape(OUT_CH, IMGS, 256) for c in range(N_CORES)], axis=1
    )  # (64, B, 256) sum-pooled exp values, spatial (y,x) 16x16
    S1 = sum(res1[c]["s12"][:, 0] for c in range(N_CORES))
    S2 = sum(res1[c]["s12"][:, 1] for c in range(N_CORES))
    nS = B * 256
    muP = S1 / nS
    varP = S2 / nS - muP * muP
    s1f = np.asarray(bn1_g) / np.sqrt(varP + 16.0 * BN_EPS / (C1 * C1))
    t1f = np.asarray(bn1_b) - muP * s1f

    # conv2 weights: fold s1 into ic columns; bias channel = conv of t1 map
    w2 = np.asarray(conv2_w, np.float32)  # (128, 64, 3, 3)
    w2f = w2 * s1f[None, :, None, None]
    tmap = np.einsum("oikl,i->okl", w2, t1f)  # (128, 3, 3)
    w2aug = np.zeros((65, 9 * 128), np.float32)
    for tap in range(9):
        ky, kx = tap // 3, tap % 3
        w2aug[:64, tap * 128:(tap + 1) * 128] = w2f[:, :, ky, kx].T
        w2aug[64, tap * 128:(tap + 1) * 128] = tmap[:, ky, kx]
    # compact h1: rows 0-63 = h1S, row 64 = ones channel (padding done on device)
    h1n = np.ones((65, B * 256), np.float32)
    h1n[:64] = h1S.reshape(OUT_CH, B * 256)

    def taps(wc):  # (oc, ic, 3, 3) -> (ic, 9*oc)
        out = np.zeros((wc.shape[1], 9 * wc.shape[0]), np.float32)
        for tap in range(9):
            ky, kx = tap // 3, tap % 3
            out[:, tap * wc.shape[0]:(tap + 1) * wc.shape[0]] = wc[:, :, ky, kx].T
        return out

    w3t = taps(np.asarray(conv3_w, np.float32) * 0.25)
    w4t = taps(np.asarray(conv4_w, np.float32))
    bng = np.stack(
        [np.asarray(v, np.float32) for v in
         (bn2_g, bn2_b, bn3_g, bn3_b, bn4_g, bn4_b, bnfc_g, bnfc_b)], axis=1
    )  # (128, 8)

    if "nc2" not in _cache:
        _cache["nc2"] = _build_phase2()
    nc2 = _cache["nc2"]
    im2 = {
        "h1n": h1n,
        "w2": w2aug,
        "w3": w3t,
        "w4": w4t,
        "fc1": np.asarray(fc1_w, np.float32).reshape(128, 128).T.copy(),
        "fc2": np.asarray(fc2_w, np.float32).reshape(10, 128).T.copy(),
        "bng": bng,
    }
    res2 = run_bass_kernel_spmd(nc2, [im2], [0]).results

    xm = res2[0]["xm"].T.copy() + np.asarray(fc2_b, np.float32)[None, :]  # (B, 10)
    xe = res2[0]["xe"].T.reshape(B, 128, 1, 1).copy()
    return xm, xe
